# revision 7
# baseline (speedup 1.0000x reference)
"""GAT-style message passing kernel for Trainium2 (8 NeuronCores, SPMD).

h_prime[i] = (sum_j exp(lrelu(<label_i,label_j>)) * h[j]) / rowsum_i @ W

Algebraic transform: W commutes with the segment-sum, so raw h[dst] rows are
aggregated per src node and W applied once per 128-row block.  Per 128-edge
tile a selector matmul (S[p,q] = (q==srcl_p) * exp_p) computes the weighted
sum; a ones column appended to the gathered rows yields the softmax row-sums
in the same matmul.

v2 data path: edge-endpoint rows are fetched with dma_gather (one GPSIMD
instruction per (4-block group, dst-range) instead of one indirect DMA per
tile), from a packed bf16 table row [label(f32-bitcast) | h(bf16) | 1 | pad]
of 768 bytes.  dst ranges of 32768 rows keep gather indices within int16.
Edge src labels are streamed as a host-prepared per-slot array via plain
HWDGE DMA.  Attention logits are computed with DVE ops batched over a whole
gather's tiles.

Sharding: nodes (src) split 12500/core; each core gets all edges whose src it
owns.  One NEFF runs SPMD on all 8 cores; the slot schedule is the per-
(block,range) max across cores so the program is identical.
"""

import sys

sys.path.insert(0, "/opt/trn_rl_repo")

import json

import numpy as np
import ml_dtypes

import concourse.bass as bass
import concourse.mybir as mybir
from concourse import library_config
from concourse.bass_utils import run_bass_kernel_spmd
from concourse.library_overlay import lower_extended_insts

from concourse.tile import TileContext


def _legalize_waits(bir: bytes) -> bytes:
    """This toolchain's codegen allows one sync-wait per instruction; move
    extras onto injected wait-only EventSemaphore ops in the same queue."""
    d = json.loads(bir)
    n = 0
    for fn in d["functions"]:
        for blk in fn["blocks"]:
            out = []
            for inst in blk["instructions"]:
                si = inst.get("sync_info")
                ow = (si or {}).get("on_wait") or []
                if len(ow) > 1:
                    for w in ow[:-1]:
                        n += 1
                        out.append(
                            {
                                "debug": inst.get("debug", 0),
                                "engine": inst.get("engine"),
                                "ins": [],
                                "name": f"waitfix_{n}_{inst['name']}",
                                "opcode": "EventSemaphore",
                                "outs": [],
                                "sync_info": {"on_update": [], "on_wait": [w]},
                            }
                        )
                    si["on_wait"] = [ow[-1]]
                out.append(inst)
            blk["instructions"] = out
    return json.dumps(d).encode()


_orig_to_json_bytes = bass.Bass.to_json_bytes


def _patched_to_json_bytes(self):
    return _legalize_waits(_orig_to_json_bytes(self))


bass.Bass.to_json_bytes = _patched_to_json_bytes

N = 100000
E = 1600000
IN_F = 256
D_LABEL = 32
OUT_F = 256
ALPHA = 0.2
EPS = 1e-9
NCORES = 8
SHARD = N // NCORES          # 12500
BLK = 128
NBLK = (SHARD + BLK - 1) // BLK   # 98
RNG = 32768                  # dst range size (int16 gather indices)
NRANGE = (N + RNG - 1) // RNG     # 4
GRP = 4                      # blocks per gather group
NGRP = (NBLK + GRP - 1) // GRP    # 25
ROW = 384                    # bf16 row: 64 lab(f32 bitcast) | 256 h | 1 | pad

F32 = mybir.dt.float32
BF16 = mybir.dt.bfloat16
I16 = mybir.dt.int16


def _host_prep(h, label, W, adj_indices):
    src = np.asarray(adj_indices[0], dtype=np.int64)
    dst = np.asarray(adj_indices[1], dtype=np.int64)

    # packed bf16 table row: [label.bitcast | h.bf16 | 1 | pad] = 768 B
    tab = np.zeros((N, ROW), dtype=ml_dtypes.bfloat16)
    tab[:, 0 : 2 * D_LABEL] = (
        np.ascontiguousarray(label, dtype=np.float32)
        .view(ml_dtypes.bfloat16)
        .reshape(N, 2 * D_LABEL)
    )
    tab[:, 2 * D_LABEL : 2 * D_LABEL + IN_F] = h.astype(ml_dtypes.bfloat16)
    tab[:, 2 * D_LABEL + IN_F] = 1.0

    # sort edges once by (core, block, range, dst)
    core = src // SHARD
    blk = (src % SHARD) // BLK
    rng_id = dst // RNG
    key = ((core * NBLK + blk) * NRANGE + rng_id) * np.int64(N) + dst
    order = np.argsort(key, kind="stable")
    s_s, d_s = src[order], dst[order]
    core_s, blk_s, rng_s = core[order], blk[order], rng_id[order]

    # shared schedule: tiles per (block, range) = max over cores
    counts = np.zeros((NCORES, NBLK, NRANGE), dtype=np.int64)
    np.add.at(counts, (core_s, blk_s, rng_s), 1)
    tiles = ((counts + BLK - 1) // BLK).max(axis=0)  # [NBLK, NRANGE]

    # global tile order: (group, range, block in group, tile)
    # tile_start[b, r] = global tile index of (b, r)'s first tile
    tile_start = np.zeros((NBLK, NRANGE), dtype=np.int64)
    t = 0
    sched = []  # per group: (blocks, [(r, [(b, t0, ntile)...])...])
    for g in range(NGRP):
        b0, b1 = g * GRP, min((g + 1) * GRP, NBLK)
        per_r = []
        for r in range(NRANGE):
            ents = []
            for b in range(b0, b1):
                nt = int(tiles[b, r])
                if nt == 0:
                    continue
                tile_start[b, r] = t
                ents.append((b, t, nt))
                t += nt
            per_r.append(ents)
        sched.append((list(range(b0, b1)), per_r))
    T_total = t

    # per-core slot arrays
    C_total = T_total * BLK // 16
    Wt = np.ascontiguousarray(
        np.concatenate([W[:128, :], W[128:, :]], axis=1), dtype=np.float32
    )
    iota_f = np.tile(np.arange(128, dtype=np.float32), (128, 1))
    ident = np.eye(128, dtype=np.float32)

    # per (core, block, range) run boundaries in the sorted edge list
    run_key = (core_s * NBLK + blk_s) * NRANGE + rng_s
    bounds = np.searchsorted(run_key, np.arange(NCORES * NBLK * NRANGE + 1))

    in_maps = []
    for m in range(NCORES):
        idx16 = np.zeros((128, C_total), dtype=np.int16)
        srcl = np.full((128, T_total), 300.0, dtype=np.float32)
        lsre = np.zeros((128, T_total * D_LABEL), dtype=np.float32)

        # slot index for every real edge of this core
        for b in range(NBLK):
            for r in range(NRANGE):
                k = (m * NBLK + b) * NRANGE + r
                e0, e1 = bounds[k], bounds[k + 1]
                if e0 == e1:
                    continue
                n_e = e1 - e0
                s0 = tile_start[b, r] * BLK
                sl = s0 + np.arange(n_e)
                dst_loc = (d_s[e0:e1] - r * RNG).astype(np.int16)
                idx16[sl % 16, sl // 16] = dst_loc
                srcl[sl % BLK, sl // BLK] = (s_s[e0:e1] % SHARD - b * BLK).astype(
                    np.float32
                )
                lab_rows = label[s_s[e0:e1]]
                p = sl % BLK
                c0 = (sl // BLK) * D_LABEL
                # scatter rows: lsre[p, c0:c0+32] = lab_rows
                flat_cols = c0[:, None] + np.arange(D_LABEL)[None, :]
                lsre[p[:, None], flat_cols] = lab_rows
        idx16[16:32, :] = idx16[:16, :]

        in_maps.append(
            {
                "tab": tab,
                "idx": idx16,
                "srcl": srcl,
                "lsre": lsre,
                "wt": Wt,
                "iotaf": iota_f,
                "identf": ident,
            }
        )

    sched_key = tiles.tobytes()
    return in_maps, sched, T_total, sched_key


def _build_kernel(sched, T_total):
    nc = bass.Bass()

    C_total = T_total * BLK // 16
    tab_d = nc.dram_tensor("tab", [N, ROW], BF16, kind="ExternalInput")
    idx_d = nc.dram_tensor("idx", [128, C_total], I16, kind="ExternalInput")
    srcl_d = nc.dram_tensor("srcl", [128, T_total], F32, kind="ExternalInput")
    lsre_d = nc.dram_tensor(
        "lsre", [128, T_total * D_LABEL], F32, kind="ExternalInput"
    )
    wt_d = nc.dram_tensor("wt", [128, 2 * OUT_F], F32, kind="ExternalInput")
    iota_d = nc.dram_tensor("iotaf", [128, 128], F32, kind="ExternalInput")
    ident_d = nc.dram_tensor("identf", [128, 128], F32, kind="ExternalInput")
    out_d = nc.dram_tensor("out", [SHARD, OUT_F], F32, kind="ExternalOutput")

    with TileContext(nc) as tc:
        with (
            tc.tile_pool(name="const", bufs=1) as cpool,
            tc.tile_pool(name="g0", bufs=2) as gpool0,
            tc.tile_pool(name="g1", bufs=2) as gpool1,
            tc.tile_pool(name="g2", bufs=2) as gpool2,
            tc.tile_pool(name="g3", bufs=2) as gpool3,
            tc.tile_pool(name="lsr", bufs=2) as lpool,
            tc.tile_pool(name="prod", bufs=3) as ppool,
            tc.tile_pool(name="dots", bufs=10) as dpool,
            tc.tile_pool(name="sel", bufs=6) as spool,
            tc.tile_pool(name="small", bufs=6) as smpool,
            tc.tile_pool(name="post", bufs=3) as postpool,
            tc.tile_pool(name="psA", bufs=4, space="PSUM") as psA,
            tc.tile_pool(name="psT", bufs=2, space="PSUM") as psT,
            tc.tile_pool(name="psO", bufs=2, space="PSUM") as psO,
        ):
            gpools = [gpool0, gpool1, gpool2, gpool3]
            nc.gpsimd.load_library(library_config.mlp)

            # one Pool register per distinct num_idxs value (54 regs total)
            _nreg = {}

            def numreg(v):
                if v not in _nreg:
                    r = nc.gpsimd.alloc_register(f"ni_{v}")
                    nc.gpsimd.reg_add(r, 0, v)
                    _nreg[v] = r
                return _nreg[v]

            iota_f = cpool.tile([128, 128], F32, tag="iota_f")
            nc.sync.dma_start(out=iota_f[:], in_=iota_d[:, :])
            ident = cpool.tile([128, 128], F32, tag="ident")
            nc.sync.dma_start(out=ident[:], in_=ident_d[:, :])
            wt_sb = cpool.tile([128, 2 * OUT_F], F32, tag="wt")
            nc.sync.dma_start(out=wt_sb[:], in_=wt_d[:, :])
            srcl_sb = cpool.tile([128, T_total], F32, tag="srcl")
            nc.sync.dma_start(out=srcl_sb[:], in_=srcl_d[:, :])
            idx_sb = cpool.tile([128, C_total], I16, tag="idx")
            nc.sync.dma_start(out=idx_sb[:], in_=idx_d[:, :])

            for blocks, per_r in sched:
                # group tile span
                t_g0 = per_r[0][0][1] if per_r[0] else None
                ents_all = [e for ents in per_r for e in ents]
                if not ents_all:
                    continue
                t_g0 = min(e[1] for e in ents_all)
                t_g1 = max(e[1] + e[2] for e in ents_all)
                Tg = t_g1 - t_g0

                lsre = lpool.tile([128, Tg * D_LABEL], F32, tag="lsre")
                nc.sync.dma_start(
                    out=lsre[:],
                    in_=lsre_d[:, t_g0 * D_LABEL : t_g1 * D_LABEL],
                )

                gath = [None] * NRANGE
                expv = [None] * NRANGE
                tr0 = [0] * NRANGE
                for r in range(NRANGE):
                    ents = per_r[r]
                    if not ents:
                        continue
                    T_gr = sum(e[2] for e in ents)
                    tr0[r] = ents[0][1]
                    g = gpools[r].tile([128, T_gr * ROW], BF16, tag=f"gath{r}")
                    gath[r] = (g, T_gr)
                    rbase = r * RNG
                    rend = min(N, (r + 1) * RNG)
                    nc.gpsimd.dma_gather(
                        g[:].rearrange("p (t e) -> p t e", e=ROW),
                        tab_d[rbase:rend, :],
                        idx_sb[:, tr0[r] * 8 : (tr0[r] + T_gr) * 8],
                        T_gr * BLK,
                        numreg(T_gr * BLK),
                        ROW,
                        # >64 descs per engine overflow the single-packet
                        # coalescing limit and wedge the device
                        single_packet=False,
                    )

                # batched attention logits per range
                for r in range(NRANGE):
                    if gath[r] is None:
                        continue
                    g, T_gr = gath[r]
                    lab_view = (
                        g[:, : T_gr * ROW]
                        .bitcast(F32)
                        .rearrange("p (t k) -> p t k", k=ROW // 2)[
                            :, :, 0:D_LABEL
                        ]
                    )
                    loff = (tr0[r] - t_g0) * D_LABEL
                    lsr_view = lsre[
                        :, loff : loff + T_gr * D_LABEL
                    ].rearrange("p (t k) -> p t k", k=D_LABEL)
                    prod = ppool.tile([128, T_gr * D_LABEL], F32, tag="prod")
                    nc.vector.tensor_tensor(
                        out=prod[:],
                        in0=lab_view,
                        in1=lsr_view,
                        op=mybir.AluOpType.mult,
                    )
                    dots = dpool.tile([128, T_gr], F32, tag="dots")
                    nc.vector.tensor_reduce(
                        out=dots[:],
                        in_=prod[:].rearrange("p (t k) -> p t k", k=D_LABEL),
                        axis=mybir.AxisListType.X,
                        op=mybir.AluOpType.add,
                    )
                    sc = dpool.tile([128, T_gr], F32, tag="sc")
                    nc.vector.tensor_scalar_mul(sc[:], dots[:], ALPHA)
                    lr = dpool.tile([128, T_gr], F32, tag="lr")
                    nc.vector.tensor_tensor(
                        out=lr[:],
                        in0=sc[:],
                        in1=dots[:],
                        op=mybir.AluOpType.max,
                    )
                    ev = dpool.tile([128, T_gr], F32, tag="expv")
                    nc.scalar.activation(
                        ev[:], lr[:], mybir.ActivationFunctionType.Exp
                    )
                    expv[r] = ev

                # selector matmuls, accumulated per block
                agg = {}
                first = {b: True for b in blocks}
                last_tile = {}
                for r in range(NRANGE):
                    for b, tb0, nt in per_r[r]:
                        last_tile[b] = (r, tb0 + nt - 1)
                for r in range(NRANGE):
                    if gath[r] is None:
                        continue
                    g, T_gr = gath[r]
                    for b, tb0, nt in per_r[r]:
                        if first.get(b, True):
                            agg[b] = psA.tile(
                                [128, IN_F + 1], F32, tag="agg", name=f"agg{b}"
                            )
                        for j in range(nt):
                            t = tb0 + j
                            S = spool.tile([128, 128], BF16, tag="S")
                            nc.vector.tensor_scalar(
                                out=S[:],
                                in0=iota_f[:],
                                scalar1=srcl_sb[:, t : t + 1],
                                scalar2=expv[r][:, t - tr0[r] : t - tr0[r] + 1],
                                op0=mybir.AluOpType.is_equal,
                                op1=mybir.AluOpType.mult,
                            )
                            loc = (t - tr0[r]) * ROW
                            nc.tensor.matmul(
                                out=agg[b][:],
                                lhsT=S[:],
                                rhs=g[:, loc + 2 * D_LABEL : loc + 2 * D_LABEL + IN_F + 1],
                                start=first.get(b, True),
                                stop=(last_tile[b] == (r, t)),
                            )
                            first[b] = False

                # per-block epilogue: normalize + project
                for b in blocks:
                    if b not in agg:
                        continue
                    rows = min(BLK, SHARD - b * BLK)
                    rsm = smpool.tile([128, 1], F32, tag="rsm")
                    nc.vector.tensor_scalar_max(
                        rsm[:], agg[b][:, IN_F : IN_F + 1], EPS
                    )
                    rcp = smpool.tile([128, 1], F32, tag="rcp")
                    nc.vector.reciprocal(rcp[:], rsm[:])
                    scaled = postpool.tile([128, IN_F], F32, tag="scaled")
                    nc.vector.tensor_scalar_mul(
                        scaled[:], agg[b][:, :IN_F], rcp[:]
                    )
                    outp = psO.tile([128, OUT_F], F32, tag="outp")
                    for c in range(2):
                        tp = psT.tile([128, 128], F32, tag="tp")
                        nc.tensor.transpose(
                            out=tp[:],
                            in_=scaled[:, c * 128 : (c + 1) * 128],
                            identity=ident[:],
                        )
                        sT = postpool.tile([128, 128], F32, tag="sT")
                        nc.vector.tensor_copy(sT[:], tp[:])
                        nc.tensor.matmul(
                            out=outp[:],
                            lhsT=sT[:],
                            rhs=wt_sb[:, c * OUT_F : (c + 1) * OUT_F],
                            start=(c == 0),
                            stop=(c == 1),
                        )
                    osb = postpool.tile([128, OUT_F], F32, tag="osb")
                    nc.vector.tensor_copy(osb[:], outp[:])
                    nc.sync.dma_start(
                        out=out_d[b * BLK : b * BLK + rows, :], in_=osb[:rows, :]
                    )

    lower_extended_insts(nc)
    return nc


_CACHE = {}


def kernel(h, label, W, adj_indices):
    h = np.asarray(h, dtype=np.float32)
    label = np.asarray(label, dtype=np.float32)
    W = np.asarray(W, dtype=np.float32)
    adj_indices = np.asarray(adj_indices)

    in_maps, sched, T_total, sched_key = _host_prep(h, label, W, adj_indices)

    if sched_key not in _CACHE:
        _CACHE[sched_key] = _build_kernel(sched, T_total)
    nc = _CACHE[sched_key]

    res = run_bass_kernel_spmd(nc, in_maps, core_ids=list(range(NCORES)))
    out = np.concatenate([r["out"] for r in res.results], axis=0)
    return out.astype(np.float32)


# revision 17
# speedup vs baseline: 1.2830x; 1.2830x over previous
"""GAT-style message passing kernel for Trainium2 (8 NeuronCores, SPMD).

h_prime[i] = (sum_j exp(lrelu(<label_i,label_j>)) * h[j]) / rowsum_i @ W

Algebraic transform: W commutes with the segment-sum, so raw h[dst] rows are
aggregated per src node and W applied once per 128-row block.  Per 128-edge
tile a selector matmul (S[p,q] = (q==srcl_p) * exp_p) computes the weighted
sum; a ones column appended to the gathered rows yields the softmax row-sums
in the same matmul.

v2 data path: edge-endpoint rows are fetched with dma_gather (one GPSIMD
instruction per (4-block group, dst-range) instead of one indirect DMA per
tile), from a packed bf16 table row [label(f32-bitcast) | h(bf16) | 1 | pad]
of 768 bytes.  dst ranges of 32768 rows keep gather indices within int16.
Edge src labels are streamed as a host-prepared per-slot array via plain
HWDGE DMA.  Attention logits are computed with DVE ops batched over a whole
gather's tiles.

Sharding: nodes (src) split 12500/core; each core gets all edges whose src it
owns.  One NEFF runs SPMD on all 8 cores; the slot schedule is the per-
(block,range) max across cores so the program is identical.
"""

import sys

sys.path.insert(0, "/opt/trn_rl_repo")

import json

import numpy as np
import ml_dtypes

import concourse.bass as bass
import concourse.mybir as mybir
from concourse import library_config
from concourse.bass_utils import run_bass_kernel_spmd
from concourse.library_overlay import lower_extended_insts

from concourse.tile import TileContext


def _legalize_waits(bir: bytes) -> bytes:
    """This toolchain's codegen allows one sync-wait per instruction; move
    extras onto injected wait-only EventSemaphore ops in the same queue."""
    d = json.loads(bir)
    n = 0
    for fn in d["functions"]:
        for blk in fn["blocks"]:
            out = []
            for inst in blk["instructions"]:
                si = inst.get("sync_info")
                ow = (si or {}).get("on_wait") or []
                if len(ow) > 1:
                    for w in ow[:-1]:
                        n += 1
                        out.append(
                            {
                                "debug": inst.get("debug", 0),
                                "engine": inst.get("engine"),
                                "ins": [],
                                "name": f"waitfix_{n}_{inst['name']}",
                                "opcode": "EventSemaphore",
                                "outs": [],
                                "sync_info": {"on_update": [], "on_wait": [w]},
                            }
                        )
                    si["on_wait"] = [ow[-1]]
                out.append(inst)
            blk["instructions"] = out
    return json.dumps(d).encode()


_orig_to_json_bytes = bass.Bass.to_json_bytes


def _patched_to_json_bytes(self):
    return _legalize_waits(_orig_to_json_bytes(self))


bass.Bass.to_json_bytes = _patched_to_json_bytes

N = 100000
E = 1600000
IN_F = 256
D_LABEL = 32
OUT_F = 256
ALPHA = 0.2
EPS = 1e-9
NCORES = 8
SHARD = N // NCORES          # 12500
BLK = 128
NBLK = (SHARD + BLK - 1) // BLK   # 98
RNG = 32768                  # dst range size (int16 gather indices)
NRANGE = (N + RNG - 1) // RNG     # 4
GRP = 4                      # blocks per gather group
NGRP = (NBLK + GRP - 1) // GRP    # 25
ROW = 384                    # bf16 row: 64 lab(f32 bitcast) | 256 h | 1 | pad

F32 = mybir.dt.float32
BF16 = mybir.dt.bfloat16
I16 = mybir.dt.int16


def _host_prep(h, label, W, adj_indices):
    src = np.asarray(adj_indices[0], dtype=np.int64)
    dst = np.asarray(adj_indices[1], dtype=np.int64)

    # packed bf16 table row: [label.bitcast | h.bf16 | 1 | pad] = 768 B
    tab = np.zeros((N, ROW), dtype=ml_dtypes.bfloat16)
    tab[:, 0 : 2 * D_LABEL] = (
        np.ascontiguousarray(label, dtype=np.float32)
        .view(ml_dtypes.bfloat16)
        .reshape(N, 2 * D_LABEL)
    )
    tab[:, 2 * D_LABEL : 2 * D_LABEL + IN_F] = h.astype(ml_dtypes.bfloat16)
    tab[:, 2 * D_LABEL + IN_F] = 1.0

    # sort edges once by (core, block, range, dst)
    core = src // SHARD
    blk = (src % SHARD) // BLK
    rng_id = dst // RNG
    key = ((core * NBLK + blk) * NRANGE + rng_id) * np.int64(N) + dst
    order = np.argsort(key, kind="stable")
    s_s, d_s = src[order], dst[order]
    core_s, blk_s, rng_s = core[order], blk[order], rng_id[order]

    # shared schedule: tiles per (block, range) = max over cores
    counts = np.zeros((NCORES, NBLK, NRANGE), dtype=np.int64)
    np.add.at(counts, (core_s, blk_s, rng_s), 1)
    tiles = ((counts + BLK - 1) // BLK).max(axis=0)  # [NBLK, NRANGE]

    # global tile order: (group, range, block in group, tile)
    # tile_start[b, r] = global tile index of (b, r)'s first tile
    tile_start = np.zeros((NBLK, NRANGE), dtype=np.int64)
    t = 0
    sched = []  # per group: (blocks, [(r, [(b, t0, ntile)...])...])
    for g in range(NGRP):
        b0, b1 = g * GRP, min((g + 1) * GRP, NBLK)
        per_r = []
        for r in range(NRANGE):
            ents = []
            for b in range(b0, b1):
                nt = int(tiles[b, r])
                if nt == 0:
                    continue
                tile_start[b, r] = t
                ents.append((b, t, nt))
                t += nt
            per_r.append(ents)
        sched.append((list(range(b0, b1)), per_r))
    T_total = t

    # per-core slot arrays
    C_total = T_total * BLK // 16
    Wt = np.ascontiguousarray(
        np.concatenate([W[:128, :], W[128:, :]], axis=1), dtype=np.float32
    )
    iota_b = np.tile(
        np.arange(128, dtype=np.float32).astype(ml_dtypes.bfloat16), (128, 1)
    )
    ident = np.eye(128, dtype=np.float32)

    # per (core, block, range) run boundaries in the sorted edge list
    run_key = (core_s * NBLK + blk_s) * NRANGE + rng_s
    bounds = np.searchsorted(run_key, np.arange(NCORES * NBLK * NRANGE + 1))

    in_maps = []
    for m in range(NCORES):
        idx16 = np.zeros((128, C_total), dtype=np.int16)
        srcl = np.full((128, T_total), 300.0, dtype=np.float32)
        lsre = np.zeros((128, T_total * D_LABEL), dtype=np.float32)

        # slot index for every real edge of this core
        for b in range(NBLK):
            for r in range(NRANGE):
                k = (m * NBLK + b) * NRANGE + r
                e0, e1 = bounds[k], bounds[k + 1]
                if e0 == e1:
                    continue
                n_e = e1 - e0
                s0 = tile_start[b, r] * BLK
                sl = s0 + np.arange(n_e)
                dst_loc = (d_s[e0:e1] - r * RNG).astype(np.int16)
                idx16[sl % 16, sl // 16] = dst_loc
                srcl[sl % BLK, sl // BLK] = (s_s[e0:e1] % SHARD - b * BLK).astype(
                    np.float32
                )
                lab_rows = label[s_s[e0:e1]]
                p = sl % BLK
                c0 = (sl // BLK) * D_LABEL
                # scatter rows: lsre[p, c0:c0+32] = lab_rows
                flat_cols = c0[:, None] + np.arange(D_LABEL)[None, :]
                lsre[p[:, None], flat_cols] = lab_rows
        idx16[16:32, :] = idx16[:16, :]

        in_maps.append(
            {
                "tab": tab,
                "idx": idx16,
                "srcl": srcl.astype(ml_dtypes.bfloat16),
                "lsre": lsre,
                "wt": Wt,
                "iotab": iota_b,
                "identf": ident,
            }
        )

    sched_key = tiles.tobytes()
    return in_maps, sched, T_total, sched_key


def _build_kernel(sched, T_total):
    nc = bass.Bass()

    C_total = T_total * BLK // 16
    tab_d = nc.dram_tensor("tab", [N, ROW], BF16, kind="ExternalInput")
    idx_d = nc.dram_tensor("idx", [128, C_total], I16, kind="ExternalInput")
    srcl_d = nc.dram_tensor("srcl", [128, T_total], BF16, kind="ExternalInput")
    lsre_d = nc.dram_tensor(
        "lsre", [128, T_total * D_LABEL], F32, kind="ExternalInput"
    )
    wt_d = nc.dram_tensor("wt", [128, 2 * OUT_F], F32, kind="ExternalInput")
    iota_d = nc.dram_tensor("iotab", [128, 128], BF16, kind="ExternalInput")
    ident_d = nc.dram_tensor("identf", [128, 128], F32, kind="ExternalInput")
    out_d = nc.dram_tensor("out", [SHARD, OUT_F], F32, kind="ExternalOutput")

    with TileContext(nc) as tc:
        with (
            tc.tile_pool(name="const", bufs=1) as cpool,
            tc.tile_pool(name="g0", bufs=2) as gpool0,
            tc.tile_pool(name="g1", bufs=2) as gpool1,
            tc.tile_pool(name="g2", bufs=2) as gpool2,
            tc.tile_pool(name="g3", bufs=2) as gpool3,
            tc.tile_pool(name="lsr", bufs=2) as lpool,
            tc.tile_pool(name="idxp", bufs=2) as idxpool,
            tc.tile_pool(name="prod", bufs=3) as ppool,
            tc.tile_pool(name="dots", bufs=10) as dpool,
            tc.tile_pool(name="sel", bufs=3) as spool,
            tc.tile_pool(name="small", bufs=6) as smpool,
            tc.tile_pool(name="post", bufs=3) as postpool,
            tc.tile_pool(name="psA", bufs=4, space="PSUM") as psA,
            tc.tile_pool(name="psT", bufs=2, space="PSUM") as psT,
            tc.tile_pool(name="psO", bufs=2, space="PSUM") as psO,
        ):
            gpools = [gpool0, gpool1, gpool2, gpool3]
            nc.gpsimd.load_library(library_config.mlp)

            # one Pool register per distinct num_idxs value (54 regs total)
            _nreg = {}

            def numreg(v):
                if v not in _nreg:
                    r = nc.gpsimd.alloc_register(f"ni_{v}")
                    nc.gpsimd.reg_add(r, 0, v)
                    _nreg[v] = r
                return _nreg[v]

            iota_b = cpool.tile([128, 128], BF16, tag="iota_b")
            nc.sync.dma_start(out=iota_b[:], in_=iota_d[:, :])
            ident = cpool.tile([128, 128], F32, tag="ident")
            nc.sync.dma_start(out=ident[:], in_=ident_d[:, :])
            wt_sb = cpool.tile([128, 2 * OUT_F], F32, tag="wt")
            nc.sync.dma_start(out=wt_sb[:], in_=wt_d[:, :])
            srcl_sb = cpool.tile([128, T_total], BF16, tag="srcl")
            nc.sync.dma_start(out=srcl_sb[:], in_=srcl_d[:, :])

            for blocks, per_r in sched:
                # group tile span
                t_g0 = per_r[0][0][1] if per_r[0] else None
                ents_all = [e for ents in per_r for e in ents]
                if not ents_all:
                    continue
                t_g0 = min(e[1] for e in ents_all)
                t_g1 = max(e[1] + e[2] for e in ents_all)
                Tg = t_g1 - t_g0

                lsre = lpool.tile([128, Tg * D_LABEL], F32, tag="lsre")
                nc.sync.dma_start(
                    out=lsre[:],
                    in_=lsre_d[:, t_g0 * D_LABEL : t_g1 * D_LABEL],
                )
                idx_g = idxpool.tile([128, Tg * 8], I16, tag="idx_g")
                nc.sync.dma_start(
                    out=idx_g[:], in_=idx_d[:, t_g0 * 8 : t_g1 * 8]
                )

                gath = [None] * NRANGE
                expv = [None] * NRANGE
                tr0 = [0] * NRANGE
                for r in range(NRANGE):
                    ents = per_r[r]
                    if not ents:
                        continue
                    T_gr = sum(e[2] for e in ents)
                    tr0[r] = ents[0][1]
                    g = gpools[r].tile([128, T_gr * ROW], BF16, tag=f"gath{r}")
                    gath[r] = (g, T_gr)
                    rbase = r * RNG
                    rend = min(N, (r + 1) * RNG)
                    c0 = (tr0[r] - t_g0) * 8
                    nc.gpsimd.dma_gather(
                        g[:].rearrange("p (t e) -> p t e", e=ROW),
                        tab_d[rbase:rend, :],
                        idx_g[:, c0 : c0 + T_gr * 8],
                        T_gr * BLK,
                        numreg(T_gr * BLK),
                        ROW,
                        # >64 descs per engine overflow the single-packet
                        # coalescing limit and wedge the device
                        single_packet=False,
                    )

                # batched attention logits per range
                for r in range(NRANGE):
                    if gath[r] is None:
                        continue
                    g, T_gr = gath[r]
                    lab_view = (
                        g[:, : T_gr * ROW]
                        .bitcast(F32)
                        .rearrange("p (t k) -> p t k", k=ROW // 2)[
                            :, :, 0:D_LABEL
                        ]
                    )
                    loff = (tr0[r] - t_g0) * D_LABEL
                    lsr_view = lsre[
                        :, loff : loff + T_gr * D_LABEL
                    ].rearrange("p (t k) -> p t k", k=D_LABEL)
                    prod = ppool.tile([128, T_gr * D_LABEL], F32, tag="prod")
                    nc.vector.tensor_tensor(
                        out=prod[:],
                        in0=lab_view,
                        in1=lsr_view,
                        op=mybir.AluOpType.mult,
                    )
                    dots = dpool.tile([128, T_gr], F32, tag="dots")
                    nc.vector.tensor_reduce(
                        out=dots[:],
                        in_=prod[:].rearrange("p (t k) -> p t k", k=D_LABEL),
                        axis=mybir.AxisListType.X,
                        op=mybir.AluOpType.add,
                    )
                    sc = dpool.tile([128, T_gr], F32, tag="sc")
                    nc.vector.tensor_scalar_mul(sc[:], dots[:], ALPHA)
                    lr = dpool.tile([128, T_gr], F32, tag="lr")
                    nc.vector.tensor_tensor(
                        out=lr[:],
                        in0=sc[:],
                        in1=dots[:],
                        op=mybir.AluOpType.max,
                    )
                    ev = dpool.tile([128, T_gr], BF16, tag="expv")
                    nc.scalar.activation(
                        ev[:], lr[:], mybir.ActivationFunctionType.Exp
                    )
                    expv[r] = ev

                # batched selector build per range: S[p, t*128+q] =
                # (q == srcl[p,t]) * expv[p,t], all bf16
                sall = [None] * NRANGE
                for r in range(NRANGE):
                    if gath[r] is None:
                        continue
                    _, T_gr = gath[r]
                    S_all = spool.tile([128, T_gr * 128], BF16, tag="S_all")
                    iota_bc = (
                        iota_b[:]
                        .rearrange("p (o q) -> p o q", o=1)
                        .to_broadcast([128, T_gr, 128])
                    )
                    srcl_bc = (
                        srcl_sb[:, tr0[r] : tr0[r] + T_gr]
                        .rearrange("p (t o) -> p t o", o=1)
                        .to_broadcast([128, T_gr, 128])
                    )
                    nc.vector.tensor_tensor(
                        out=S_all[:],
                        in0=iota_bc,
                        in1=srcl_bc,
                        op=mybir.AluOpType.is_equal,
                    )
                    ev_bc = (
                        expv[r][:]
                        .rearrange("p (t o) -> p t o", o=1)
                        .to_broadcast([128, T_gr, 128])
                    )
                    nc.vector.tensor_tensor(
                        out=S_all[:],
                        in0=S_all[:].rearrange("p (t q) -> p t q", q=128),
                        in1=ev_bc,
                        op=mybir.AluOpType.mult,
                    )
                    sall[r] = S_all

                # selector matmuls, accumulated per block
                agg = {}
                first = {b: True for b in blocks}
                last_tile = {}
                for r in range(NRANGE):
                    for b, tb0, nt in per_r[r]:
                        last_tile[b] = (r, tb0 + nt - 1)
                for r in range(NRANGE):
                    if gath[r] is None:
                        continue
                    g, T_gr = gath[r]
                    for b, tb0, nt in per_r[r]:
                        if first.get(b, True):
                            agg[b] = psA.tile(
                                [128, IN_F + 1], F32, tag="agg", name=f"agg{b}"
                            )
                        for j in range(nt):
                            t = tb0 + j
                            jj = t - tr0[r]
                            loc = jj * ROW
                            nc.tensor.matmul(
                                out=agg[b][:],
                                lhsT=sall[r][:, jj * 128 : (jj + 1) * 128],
                                rhs=g[:, loc + 2 * D_LABEL : loc + 2 * D_LABEL + IN_F + 1],
                                start=first.get(b, True),
                                stop=(last_tile[b] == (r, t)),
                            )
                            first[b] = False

                # per-block epilogue: normalize + project
                for b in blocks:
                    if b not in agg:
                        continue
                    rows = min(BLK, SHARD - b * BLK)
                    rsm = smpool.tile([128, 1], F32, tag="rsm")
                    nc.vector.tensor_scalar_max(
                        rsm[:], agg[b][:, IN_F : IN_F + 1], EPS
                    )
                    rcp = smpool.tile([128, 1], F32, tag="rcp")
                    nc.vector.reciprocal(rcp[:], rsm[:])
                    scaled = postpool.tile([128, IN_F], F32, tag="scaled")
                    nc.scalar.activation(
                        scaled[:],
                        agg[b][:, :IN_F],
                        mybir.ActivationFunctionType.Copy,
                        scale=rcp[:, 0:1],
                    )
                    outp = psO.tile([128, OUT_F], F32, tag="outp")
                    for c in range(2):
                        tp = psT.tile([128, 128], F32, tag="tp")
                        nc.tensor.transpose(
                            out=tp[:],
                            in_=scaled[:, c * 128 : (c + 1) * 128],
                            identity=ident[:],
                        )
                        sT = postpool.tile([128, 128], F32, tag="sT")
                        nc.scalar.activation(
                            sT[:], tp[:], mybir.ActivationFunctionType.Copy
                        )
                        nc.tensor.matmul(
                            out=outp[:],
                            lhsT=sT[:],
                            rhs=wt_sb[:, c * OUT_F : (c + 1) * OUT_F],
                            start=(c == 0),
                            stop=(c == 1),
                        )
                    osb = postpool.tile([128, OUT_F], F32, tag="osb")
                    nc.scalar.activation(
                        osb[:], outp[:], mybir.ActivationFunctionType.Copy
                    )
                    nc.sync.dma_start(
                        out=out_d[b * BLK : b * BLK + rows, :], in_=osb[:rows, :]
                    )

    lower_extended_insts(nc)
    return nc


_CACHE = {}


def kernel(h, label, W, adj_indices):
    h = np.asarray(h, dtype=np.float32)
    label = np.asarray(label, dtype=np.float32)
    W = np.asarray(W, dtype=np.float32)
    adj_indices = np.asarray(adj_indices)

    in_maps, sched, T_total, sched_key = _host_prep(h, label, W, adj_indices)

    if sched_key not in _CACHE:
        _CACHE[sched_key] = _build_kernel(sched, T_total)
    nc = _CACHE[sched_key]

    res = run_bass_kernel_spmd(nc, in_maps, core_ids=list(range(NCORES)))
    out = np.concatenate([r["out"] for r in res.results], axis=0)
    return out.astype(np.float32)


# revision 29
# speedup vs baseline: 1.3741x; 1.0710x over previous
"""GAT-style message passing kernel for Trainium2 (8 NeuronCores, SPMD).

h_prime[i] = (sum_j exp(lrelu(<label_i,label_j>)) * h[j]) / rowsum_i @ W

Algebraic transform: W commutes with the segment-sum, so raw h[dst] rows are
aggregated per src node and W applied once per 128-row block.  Per 128-edge
tile a selector matmul (S[p,q] = (q==srcl_p) * exp_p) computes the weighted
sum; a ones column appended to the gathered rows yields the softmax row-sums
in the same matmul.

v2 data path: edge-endpoint rows are fetched with dma_gather (one GPSIMD
instruction per (4-block group, dst-range) instead of one indirect DMA per
tile), from a packed bf16 table row [label(f32-bitcast) | h(bf16) | 1 | pad]
of 768 bytes.  dst ranges of 32768 rows keep gather indices within int16.
Edge src labels are streamed as a host-prepared per-slot array via plain
HWDGE DMA.  Attention logits are computed with DVE ops batched over a whole
gather's tiles.

Sharding: nodes (src) split 12500/core; each core gets all edges whose src it
owns.  One NEFF runs SPMD on all 8 cores; the slot schedule is the per-
(block,range) max across cores so the program is identical.
"""

import sys

sys.path.insert(0, "/opt/trn_rl_repo")

import json

import numpy as np
import ml_dtypes

import concourse.bass as bass
import concourse.mybir as mybir
from concourse import library_config
from concourse.bass_utils import run_bass_kernel_spmd
from concourse.library_overlay import lower_extended_insts

from concourse.tile import TileContext


def _legalize_waits(bir: bytes) -> bytes:
    """This toolchain's codegen allows one sync-wait per instruction; move
    extras onto injected wait-only EventSemaphore ops in the same queue."""
    d = json.loads(bir)
    n = 0
    for fn in d["functions"]:
        for blk in fn["blocks"]:
            out = []
            for inst in blk["instructions"]:
                si = inst.get("sync_info")
                ow = (si or {}).get("on_wait") or []
                if len(ow) > 1:
                    for w in ow[:-1]:
                        n += 1
                        out.append(
                            {
                                "debug": inst.get("debug", 0),
                                "engine": inst.get("engine"),
                                "ins": [],
                                "name": f"waitfix_{n}_{inst['name']}",
                                "opcode": "EventSemaphore",
                                "outs": [],
                                "sync_info": {"on_update": [], "on_wait": [w]},
                            }
                        )
                    si["on_wait"] = [ow[-1]]
                out.append(inst)
            blk["instructions"] = out
    return json.dumps(d).encode()


_orig_to_json_bytes = bass.Bass.to_json_bytes


def _patched_to_json_bytes(self):
    return _legalize_waits(_orig_to_json_bytes(self))


bass.Bass.to_json_bytes = _patched_to_json_bytes

N = 100000
E = 1600000
IN_F = 256
D_LABEL = 32
OUT_F = 256
ALPHA = 0.2
EPS = 1e-9
NCORES = 8
SHARD = N // NCORES          # 12500
BLK = 128
NBLK = (SHARD + BLK - 1) // BLK   # 98
RNG = 32768                  # dst range size (int16 gather indices)
NRANGE = (N + RNG - 1) // RNG     # 4
GRP = 2                      # blocks per gather group (PSUM banks: 2 agg + 2 rowsum + 2 transpose + 2 out)
NGRP = (NBLK + GRP - 1) // GRP    # 49
ROW = 256                    # bf16 gather row: h only (512 B, dma_gather-aligned)

F32 = mybir.dt.float32
BF16 = mybir.dt.bfloat16
I16 = mybir.dt.int16


def _host_prep(h, label, W, adj_indices):
    src = np.asarray(adj_indices[0], dtype=np.int64)
    dst = np.asarray(adj_indices[1], dtype=np.int64)

    # bf16 gather table: h rows only (512 B each)
    tab = np.ascontiguousarray(h.astype(ml_dtypes.bfloat16))

    # sort edges once by (core, block, range, dst)
    core = src // SHARD
    blk = (src % SHARD) // BLK
    rng_id = dst // RNG
    key = ((core * NBLK + blk) * NRANGE + rng_id) * np.int64(N) + dst
    order = np.argsort(key, kind="stable")
    s_s, d_s = src[order], dst[order]
    core_s, blk_s, rng_s = core[order], blk[order], rng_id[order]

    # shared schedule: tiles per (block, range) = max over cores
    counts = np.zeros((NCORES, NBLK, NRANGE), dtype=np.int64)
    np.add.at(counts, (core_s, blk_s, rng_s), 1)
    tiles = ((counts + BLK - 1) // BLK).max(axis=0)  # [NBLK, NRANGE]

    # global tile order: (group, range, block in group, tile)
    # tile_start[b, r] = global tile index of (b, r)'s first tile
    tile_start = np.zeros((NBLK, NRANGE), dtype=np.int64)
    t = 0
    sched = []  # per group: (blocks, [(r, [(b, t0, ntile)...])...])
    for g in range(NGRP):
        b0, b1 = g * GRP, min((g + 1) * GRP, NBLK)
        per_r = []
        for r in range(NRANGE):
            ents = []
            for b in range(b0, b1):
                nt = int(tiles[b, r])
                if nt == 0:
                    continue
                tile_start[b, r] = t
                ents.append((b, t, nt))
                t += nt
            per_r.append(ents)
        sched.append((list(range(b0, b1)), per_r))
    T_total = t

    # per-core slot arrays
    C_total = T_total * BLK // 16
    Wt = np.ascontiguousarray(
        np.concatenate([W[:128, :], W[128:, :]], axis=1), dtype=np.float32
    )
    iota_b = np.tile(
        np.arange(128, dtype=np.float32).astype(ml_dtypes.bfloat16), (128, 1)
    )
    ident = np.eye(128, dtype=np.float32)

    # per (core, block, range) run boundaries in the sorted edge list
    run_key = (core_s * NBLK + blk_s) * NRANGE + rng_s
    bounds = np.searchsorted(run_key, np.arange(NCORES * NBLK * NRANGE + 1))

    in_maps = []
    for m in range(NCORES):
        idx16 = np.zeros((128, C_total), dtype=np.int16)
        srcl = np.full((128, T_total), 300.0, dtype=np.float32)
        lsre = np.zeros((128, T_total * D_LABEL), dtype=np.float32)
        ldste = np.zeros((128, T_total * D_LABEL), dtype=np.float32)

        # slot index for every real edge of this core
        for b in range(NBLK):
            for r in range(NRANGE):
                k = (m * NBLK + b) * NRANGE + r
                e0, e1 = bounds[k], bounds[k + 1]
                if e0 == e1:
                    continue
                n_e = e1 - e0
                s0 = tile_start[b, r] * BLK
                sl = s0 + np.arange(n_e)
                dst_loc = (d_s[e0:e1] - r * RNG).astype(np.int16)
                idx16[sl % 16, sl // 16] = dst_loc
                srcl[sl % BLK, sl // BLK] = (s_s[e0:e1] % SHARD - b * BLK).astype(
                    np.float32
                )
                p = sl % BLK
                c0 = (sl // BLK) * D_LABEL
                flat_cols = c0[:, None] + np.arange(D_LABEL)[None, :]
                lsre[p[:, None], flat_cols] = label[s_s[e0:e1]]
                ldste[p[:, None], flat_cols] = label[d_s[e0:e1]]
        # replicas for the rx/tx Q7 cores of queues 0 and 1
        idx16[16:32, :] = idx16[:16, :]
        idx16[32:48, :] = idx16[:16, :]
        idx16[48:64, :] = idx16[:16, :]

        in_maps.append(
            {
                "tab": tab,
                "idx": idx16,
                "srcl": srcl.astype(ml_dtypes.bfloat16),
                "lsre": lsre,
                "ldste": ldste,
                "wt": Wt,
                "iotab": iota_b,
                "identf": ident,
                "onesb": np.ones((128, 1), dtype=ml_dtypes.bfloat16),
            }
        )

    sched_key = tiles.tobytes()
    return in_maps, sched, T_total, sched_key


def _build_kernel(sched, T_total):
    nc = bass.Bass(num_swdge_queues=2)

    C_total = T_total * BLK // 16
    tab_d = nc.dram_tensor("tab", [N, ROW], BF16, kind="ExternalInput")
    idx_d = nc.dram_tensor("idx", [128, C_total], I16, kind="ExternalInput")
    srcl_d = nc.dram_tensor("srcl", [128, T_total], BF16, kind="ExternalInput")
    lsre_d = nc.dram_tensor(
        "lsre", [128, T_total * D_LABEL], F32, kind="ExternalInput"
    )
    ldste_d = nc.dram_tensor(
        "ldste", [128, T_total * D_LABEL], F32, kind="ExternalInput"
    )
    wt_d = nc.dram_tensor("wt", [128, 2 * OUT_F], F32, kind="ExternalInput")
    iota_d = nc.dram_tensor("iotab", [128, 128], BF16, kind="ExternalInput")
    ident_d = nc.dram_tensor("identf", [128, 128], F32, kind="ExternalInput")
    ones_d = nc.dram_tensor("onesb", [128, 1], BF16, kind="ExternalInput")
    out_d = nc.dram_tensor("out", [SHARD, OUT_F], F32, kind="ExternalOutput")

    with TileContext(nc) as tc:
        with (
            tc.tile_pool(name="const", bufs=1) as cpool,
            tc.tile_pool(name="g0", bufs=2) as gpool0,
            tc.tile_pool(name="g1", bufs=2) as gpool1,
            tc.tile_pool(name="g2", bufs=2) as gpool2,
            tc.tile_pool(name="g3", bufs=2) as gpool3,
            tc.tile_pool(name="lsr", bufs=2) as lpool,
            tc.tile_pool(name="idxp", bufs=2) as idxpool,
            tc.tile_pool(name="prod", bufs=3) as ppool,
            tc.tile_pool(name="dots", bufs=10) as dpool,
            tc.tile_pool(name="sel", bufs=3) as spool,
            tc.tile_pool(name="small", bufs=6) as smpool,
            tc.tile_pool(name="post", bufs=3) as postpool,
            tc.tile_pool(name="psA", bufs=2, space="PSUM") as psA,
            tc.tile_pool(name="psR", bufs=2, space="PSUM") as psR,
            tc.tile_pool(name="psT", bufs=2, space="PSUM") as psT,
            tc.tile_pool(name="psO", bufs=2, space="PSUM") as psO,
        ):
            gpools = [gpool0, gpool1, gpool2, gpool3]
            nc.gpsimd.load_library(library_config.mlp)

            # one Pool register per distinct num_idxs value (54 regs total)
            _nreg = {}

            def numreg(v):
                if v not in _nreg:
                    r = nc.gpsimd.alloc_register(f"ni_{v}")
                    nc.gpsimd.reg_add(r, 0, v)
                    _nreg[v] = r
                return _nreg[v]

            iota_b = cpool.tile([128, 128], BF16, tag="iota_b")
            nc.sync.dma_start(out=iota_b[:], in_=iota_d[:, :])
            ident = cpool.tile([128, 128], F32, tag="ident")
            nc.sync.dma_start(out=ident[:], in_=ident_d[:, :])
            wt_sb = cpool.tile([128, 2 * OUT_F], F32, tag="wt")
            nc.sync.dma_start(out=wt_sb[:], in_=wt_d[:, :])
            srcl_sb = cpool.tile([128, T_total], BF16, tag="srcl")
            nc.sync.dma_start(out=srcl_sb[:], in_=srcl_d[:, :])
            ones_sb = cpool.tile([128, 1], BF16, tag="ones_sb")
            nc.sync.dma_start(out=ones_sb[:], in_=ones_d[:, :])

            for blocks, per_r in sched:
                # group tile span
                t_g0 = per_r[0][0][1] if per_r[0] else None
                ents_all = [e for ents in per_r for e in ents]
                if not ents_all:
                    continue
                t_g0 = min(e[1] for e in ents_all)
                t_g1 = max(e[1] + e[2] for e in ents_all)
                Tg = t_g1 - t_g0

                lsre = lpool.tile([128, Tg * D_LABEL], F32, tag="lsre")
                nc.sync.dma_start(
                    out=lsre[:],
                    in_=lsre_d[:, t_g0 * D_LABEL : t_g1 * D_LABEL],
                )
                ldste = lpool.tile([128, Tg * D_LABEL], F32, tag="ldste")
                nc.sync.dma_start(
                    out=ldste[:],
                    in_=ldste_d[:, t_g0 * D_LABEL : t_g1 * D_LABEL],
                )
                idx_g = idxpool.tile([128, Tg * 8], I16, tag="idx_g")
                nc.sync.dma_start(
                    out=idx_g[:], in_=idx_d[:, t_g0 * 8 : t_g1 * 8]
                )

                gath = [None] * NRANGE
                expv = [None] * NRANGE
                tr0 = [0] * NRANGE
                for r in range(NRANGE):
                    ents = per_r[r]
                    if not ents:
                        continue
                    T_gr = sum(e[2] for e in ents)
                    tr0[r] = ents[0][1]
                    g = gpools[r].tile([128, T_gr * ROW], BF16, tag=f"gath{r}")
                    gath[r] = (g, T_gr)
                    rbase = r * RNG
                    rend = min(N, (r + 1) * RNG)
                    c0 = (tr0[r] - t_g0) * 8
                    nc.gpsimd.dma_gather(
                        g[:].rearrange("p (t e) -> p t e", e=ROW),
                        tab_d[rbase:rend, :],
                        idx_g[:, c0 : c0 + T_gr * 8],
                        T_gr * BLK,
                        numreg(T_gr * BLK),
                        ROW,
                        # >64 descs per engine overflow the single-packet
                        # coalescing limit and wedge the device
                        single_packet=False,
                        # two queues double the in-flight descriptor rings
                        queue_num=r % 2,
                    )

                # batched attention logits per range (src/dst label edge arrays)
                for r in range(NRANGE):
                    if gath[r] is None:
                        continue
                    g, T_gr = gath[r]
                    loff = (tr0[r] - t_g0) * D_LABEL
                    prod = ppool.tile([128, T_gr * D_LABEL], F32, tag="prod")
                    nc.vector.tensor_tensor(
                        out=prod[:],
                        in0=ldste[:, loff : loff + T_gr * D_LABEL],
                        in1=lsre[:, loff : loff + T_gr * D_LABEL],
                        op=mybir.AluOpType.mult,
                    )
                    dots = dpool.tile([128, T_gr], F32, tag="dots")
                    nc.vector.tensor_reduce(
                        out=dots[:],
                        in_=prod[:].rearrange("p (t k) -> p t k", k=D_LABEL),
                        axis=mybir.AxisListType.X,
                        op=mybir.AluOpType.add,
                    )
                    sc = dpool.tile([128, T_gr], F32, tag="sc")
                    nc.vector.tensor_scalar_mul(sc[:], dots[:], ALPHA)
                    lr = dpool.tile([128, T_gr], F32, tag="lr")
                    nc.vector.tensor_tensor(
                        out=lr[:],
                        in0=sc[:],
                        in1=dots[:],
                        op=mybir.AluOpType.max,
                    )
                    ev = dpool.tile([128, T_gr], BF16, tag="expv")
                    nc.scalar.activation(
                        ev[:], lr[:], mybir.ActivationFunctionType.Exp
                    )
                    expv[r] = ev

                # batched selector build per range: S[p, t*128+q] =
                # (q == srcl[p,t]) * expv[p,t], all bf16
                sall = [None] * NRANGE
                for r in range(NRANGE):
                    if gath[r] is None:
                        continue
                    _, T_gr = gath[r]
                    S_all = spool.tile([128, T_gr * 128], BF16, tag="S_all")
                    iota_bc = (
                        iota_b[:]
                        .rearrange("p (o q) -> p o q", o=1)
                        .to_broadcast([128, T_gr, 128])
                    )
                    srcl_bc = (
                        srcl_sb[:, tr0[r] : tr0[r] + T_gr]
                        .rearrange("p (t o) -> p t o", o=1)
                        .to_broadcast([128, T_gr, 128])
                    )
                    nc.vector.tensor_tensor(
                        out=S_all[:],
                        in0=iota_bc,
                        in1=srcl_bc,
                        op=mybir.AluOpType.is_equal,
                    )
                    ev_bc = (
                        expv[r][:]
                        .rearrange("p (t o) -> p t o", o=1)
                        .to_broadcast([128, T_gr, 128])
                    )
                    nc.vector.tensor_tensor(
                        out=S_all[:],
                        in0=S_all[:].rearrange("p (t q) -> p t q", q=128),
                        in1=ev_bc,
                        op=mybir.AluOpType.mult,
                    )
                    sall[r] = S_all

                # selector matmuls, accumulated per block
                agg = {}
                rs_ps = {}
                first = {b: True for b in blocks}
                last_tile = {}
                for r in range(NRANGE):
                    for b, tb0, nt in per_r[r]:
                        last_tile[b] = (r, tb0 + nt - 1)
                for r in range(NRANGE):
                    if gath[r] is None:
                        continue
                    g, T_gr = gath[r]
                    for b, tb0, nt in per_r[r]:
                        if first.get(b, True):
                            # full-bank tiles: matmul start=True clears the
                            # whole PSUM bank, so concurrent accumulation
                            # groups need exclusive banks
                            agg[b] = psA.tile(
                                [128, 512], F32, tag="agg", name=f"agg{b}"
                            )
                            rs_ps[b] = psR.tile(
                                [128, 512], F32, tag="rs", name=f"rs{b}"
                            )
                        for j in range(nt):
                            t = tb0 + j
                            jj = t - tr0[r]
                            st = first.get(b, True)
                            sp = last_tile[b] == (r, t)
                            nc.tensor.matmul(
                                out=agg[b][:, :IN_F],
                                lhsT=sall[r][:, jj * 128 : (jj + 1) * 128],
                                rhs=g[:, jj * ROW : jj * ROW + IN_F],
                                start=st,
                                stop=sp,
                            )
                            nc.tensor.matmul(
                                out=rs_ps[b][:, 0:1],
                                lhsT=sall[r][:, jj * 128 : (jj + 1) * 128],
                                rhs=ones_sb[:],
                                start=st,
                                stop=sp,
                            )
                            first[b] = False

                # per-block epilogue: normalize + project
                for b in blocks:
                    if b not in agg:
                        continue
                    rows = min(BLK, SHARD - b * BLK)
                    rsm = smpool.tile([128, 1], F32, tag="rsm")
                    nc.vector.tensor_scalar_max(
                        rsm[:], rs_ps[b][:, 0:1], EPS
                    )
                    rcp = smpool.tile([128, 1], F32, tag="rcp")
                    nc.vector.reciprocal(rcp[:], rsm[:])
                    scaled = postpool.tile([128, IN_F], F32, tag="scaled")
                    nc.scalar.activation(
                        scaled[:],
                        agg[b][:, :IN_F],
                        mybir.ActivationFunctionType.Copy,
                        scale=rcp[:, 0:1],
                    )
                    outp = psO.tile(
                        [128, 512], F32, tag="outp", padded_shape=None
                    )
                    for c in range(2):
                        tp = psT.tile([128, 512], F32, tag="tp")
                        nc.tensor.transpose(
                            out=tp[:, 0:128],
                            in_=scaled[:, c * 128 : (c + 1) * 128],
                            identity=ident[:],
                        )
                        sT = postpool.tile([128, 128], F32, tag="sT")
                        nc.scalar.activation(
                            sT[:], tp[:, 0:128], mybir.ActivationFunctionType.Copy
                        )
                        nc.tensor.matmul(
                            out=outp[:, :OUT_F],
                            lhsT=sT[:],
                            rhs=wt_sb[:, c * OUT_F : (c + 1) * OUT_F],
                            start=(c == 0),
                            stop=(c == 1),
                        )
                    osb = postpool.tile([128, OUT_F], F32, tag="osb")
                    nc.scalar.activation(
                        osb[:], outp[:, :OUT_F], mybir.ActivationFunctionType.Copy
                    )
                    nc.sync.dma_start(
                        out=out_d[b * BLK : b * BLK + rows, :], in_=osb[:rows, :]
                    )

    lower_extended_insts(nc)
    return nc


_CACHE = {}


def kernel(h, label, W, adj_indices):
    h = np.asarray(h, dtype=np.float32)
    label = np.asarray(label, dtype=np.float32)
    W = np.asarray(W, dtype=np.float32)
    adj_indices = np.asarray(adj_indices)

    in_maps, sched, T_total, sched_key = _host_prep(h, label, W, adj_indices)

    if sched_key not in _CACHE:
        _CACHE[sched_key] = _build_kernel(sched, T_total)
    nc = _CACHE[sched_key]

    res = run_bass_kernel_spmd(nc, in_maps, core_ids=list(range(NCORES)))
    out = np.concatenate([r["out"] for r in res.results], axis=0)
    return out.astype(np.float32)


# revision 30
# speedup vs baseline: 1.5217x; 1.1074x over previous
"""GAT-style message passing kernel for Trainium2 (8 NeuronCores, SPMD).

h_prime[i] = (sum_j exp(lrelu(<label_i,label_j>)) * h[j]) / rowsum_i @ W

Algebraic transform: W commutes with the segment-sum, so raw h[dst] rows are
aggregated per src node and W applied once per 128-row block.  Per 128-edge
tile a selector matmul (S[p,q] = (q==srcl_p) * exp_p) computes the weighted
sum; a ones column appended to the gathered rows yields the softmax row-sums
in the same matmul.

v2 data path: edge-endpoint rows are fetched with dma_gather (one GPSIMD
instruction per (4-block group, dst-range) instead of one indirect DMA per
tile), from a packed bf16 table row [label(f32-bitcast) | h(bf16) | 1 | pad]
of 768 bytes.  dst ranges of 32768 rows keep gather indices within int16.
Edge src labels are streamed as a host-prepared per-slot array via plain
HWDGE DMA.  Attention logits are computed with DVE ops batched over a whole
gather's tiles.

Sharding: nodes (src) split 12500/core; each core gets all edges whose src it
owns.  One NEFF runs SPMD on all 8 cores; the slot schedule is the per-
(block,range) max across cores so the program is identical.
"""

import sys

sys.path.insert(0, "/opt/trn_rl_repo")

import json

import numpy as np
import ml_dtypes

import concourse.bass as bass
import concourse.mybir as mybir
from concourse import library_config
from concourse.bass_utils import run_bass_kernel_spmd
from concourse.library_overlay import lower_extended_insts

from concourse.tile import TileContext


def _legalize_waits(bir: bytes) -> bytes:
    """This toolchain's codegen allows one sync-wait per instruction; move
    extras onto injected wait-only EventSemaphore ops in the same queue."""
    d = json.loads(bir)
    n = 0
    for fn in d["functions"]:
        for blk in fn["blocks"]:
            out = []
            for inst in blk["instructions"]:
                si = inst.get("sync_info")
                ow = (si or {}).get("on_wait") or []
                if len(ow) > 1:
                    for w in ow[:-1]:
                        n += 1
                        out.append(
                            {
                                "debug": inst.get("debug", 0),
                                "engine": inst.get("engine"),
                                "ins": [],
                                "name": f"waitfix_{n}_{inst['name']}",
                                "opcode": "EventSemaphore",
                                "outs": [],
                                "sync_info": {"on_update": [], "on_wait": [w]},
                            }
                        )
                    si["on_wait"] = [ow[-1]]
                out.append(inst)
            blk["instructions"] = out
    return json.dumps(d).encode()


_orig_to_json_bytes = bass.Bass.to_json_bytes


def _patched_to_json_bytes(self):
    return _legalize_waits(_orig_to_json_bytes(self))


bass.Bass.to_json_bytes = _patched_to_json_bytes

N = 100000
E = 1600000
IN_F = 256
D_LABEL = 32
OUT_F = 256
ALPHA = 0.2
EPS = 1e-9
NCORES = 8
SHARD = N // NCORES          # 12500
BLK = 128
NBLK = (SHARD + BLK - 1) // BLK   # 98
RNG = 32768                  # dst range size (int16 gather indices)
NRANGE = (N + RNG - 1) // RNG     # 4
GRP = 2                      # blocks per gather group (PSUM banks: 2 agg + 2 rowsum + 2 transpose + 2 out)
NGRP = (NBLK + GRP - 1) // GRP    # 49
ROW = 256                    # bf16 gather row: h only (512 B, dma_gather-aligned)

F32 = mybir.dt.float32
BF16 = mybir.dt.bfloat16
I16 = mybir.dt.int16


def _host_prep(h, label, W, adj_indices):
    src = np.asarray(adj_indices[0], dtype=np.int64)
    dst = np.asarray(adj_indices[1], dtype=np.int64)

    # bf16 gather table: h rows only (512 B each)
    tab = np.ascontiguousarray(h.astype(ml_dtypes.bfloat16))

    # sort edges once by (core, block, range, dst)
    core = src // SHARD
    blk = (src % SHARD) // BLK
    rng_id = dst // RNG
    key = ((core * NBLK + blk) * NRANGE + rng_id) * np.int64(N) + dst
    order = np.argsort(key, kind="stable")
    s_s, d_s = src[order], dst[order]
    core_s, blk_s, rng_s = core[order], blk[order], rng_id[order]

    # shared schedule: tiles per (block, range) = max over cores
    counts = np.zeros((NCORES, NBLK, NRANGE), dtype=np.int64)
    np.add.at(counts, (core_s, blk_s, rng_s), 1)
    tiles = ((counts + BLK - 1) // BLK).max(axis=0)  # [NBLK, NRANGE]

    # global tile order: (group, range, block in group, tile)
    # tile_start[b, r] = global tile index of (b, r)'s first tile
    tile_start = np.zeros((NBLK, NRANGE), dtype=np.int64)
    t = 0
    sched = []  # per group: (blocks, [(r, [(b, t0, ntile)...])...])
    for g in range(NGRP):
        b0, b1 = g * GRP, min((g + 1) * GRP, NBLK)
        per_r = []
        for r in range(NRANGE):
            ents = []
            for b in range(b0, b1):
                nt = int(tiles[b, r])
                if nt == 0:
                    continue
                tile_start[b, r] = t
                ents.append((b, t, nt))
                t += nt
            per_r.append(ents)
        sched.append((list(range(b0, b1)), per_r))
    T_total = t

    # per-core slot arrays
    C_total = T_total * BLK // 16
    Wt = np.ascontiguousarray(
        np.concatenate([W[:128, :], W[128:, :]], axis=1), dtype=np.float32
    )
    iota_b = np.tile(
        np.arange(128, dtype=np.float32).astype(ml_dtypes.bfloat16), (128, 1)
    )
    ident = np.eye(128, dtype=np.float32)

    # per (core, block, range) run boundaries in the sorted edge list
    run_key = (core_s * NBLK + blk_s) * NRANGE + rng_s
    bounds = np.searchsorted(run_key, np.arange(NCORES * NBLK * NRANGE + 1))

    in_maps = []
    for m in range(NCORES):
        idx16 = np.zeros((128, C_total), dtype=np.int16)
        srcl = np.full((128, T_total), 300.0, dtype=np.float32)
        lsre = np.zeros((128, T_total * D_LABEL), dtype=np.float32)
        ldste = np.zeros((128, T_total * D_LABEL), dtype=np.float32)

        # slot index for every real edge of this core
        for b in range(NBLK):
            for r in range(NRANGE):
                k = (m * NBLK + b) * NRANGE + r
                e0, e1 = bounds[k], bounds[k + 1]
                if e0 == e1:
                    continue
                n_e = e1 - e0
                s0 = tile_start[b, r] * BLK
                sl = s0 + np.arange(n_e)
                dst_loc = (d_s[e0:e1] - r * RNG).astype(np.int16)
                idx16[sl % 16, sl // 16] = dst_loc
                srcl[sl % BLK, sl // BLK] = (s_s[e0:e1] % SHARD - b * BLK).astype(
                    np.float32
                )
                p = sl % BLK
                c0 = (sl // BLK) * D_LABEL
                flat_cols = c0[:, None] + np.arange(D_LABEL)[None, :]
                lsre[p[:, None], flat_cols] = label[s_s[e0:e1]]
                ldste[p[:, None], flat_cols] = label[d_s[e0:e1]]
        # replicas for the rx/tx Q7 cores of queues 0 and 1
        idx16[16:32, :] = idx16[:16, :]
        idx16[32:48, :] = idx16[:16, :]
        idx16[48:64, :] = idx16[:16, :]

        in_maps.append(
            {
                "tab": tab,
                "idx": idx16,
                "srcl": srcl.astype(ml_dtypes.bfloat16),
                "lsre": lsre,
                "ldste": ldste,
                "wt": Wt,
                "iotab": iota_b,
                "identf": ident,
                "onesb": np.ones((128, 1), dtype=ml_dtypes.bfloat16),
            }
        )

    sched_key = tiles.tobytes()
    return in_maps, sched, T_total, sched_key


def _build_kernel(sched, T_total):
    nc = bass.Bass(num_swdge_queues=2)

    C_total = T_total * BLK // 16
    tab_d = nc.dram_tensor("tab", [N, ROW], BF16, kind="ExternalInput")
    idx_d = nc.dram_tensor("idx", [128, C_total], I16, kind="ExternalInput")
    srcl_d = nc.dram_tensor("srcl", [128, T_total], BF16, kind="ExternalInput")
    lsre_d = nc.dram_tensor(
        "lsre", [128, T_total * D_LABEL], F32, kind="ExternalInput"
    )
    ldste_d = nc.dram_tensor(
        "ldste", [128, T_total * D_LABEL], F32, kind="ExternalInput"
    )
    wt_d = nc.dram_tensor("wt", [128, 2 * OUT_F], F32, kind="ExternalInput")
    iota_d = nc.dram_tensor("iotab", [128, 128], BF16, kind="ExternalInput")
    ident_d = nc.dram_tensor("identf", [128, 128], F32, kind="ExternalInput")
    ones_d = nc.dram_tensor("onesb", [128, 1], BF16, kind="ExternalInput")
    out_d = nc.dram_tensor("out", [SHARD, OUT_F], F32, kind="ExternalOutput")

    with TileContext(nc) as tc:
        with (
            tc.tile_pool(name="const", bufs=1) as cpool,
            tc.tile_pool(name="g0", bufs=2) as gpool0,
            tc.tile_pool(name="g1", bufs=2) as gpool1,
            tc.tile_pool(name="g2", bufs=2) as gpool2,
            tc.tile_pool(name="g3", bufs=2) as gpool3,
            tc.tile_pool(name="lsr", bufs=2) as lpool,
            tc.tile_pool(name="idxp", bufs=2) as idxpool,
            tc.tile_pool(name="prod", bufs=3) as ppool,
            tc.tile_pool(name="dots", bufs=10) as dpool,
            tc.tile_pool(name="sel", bufs=3) as spool,
            tc.tile_pool(name="small", bufs=6) as smpool,
            tc.tile_pool(name="post", bufs=3) as postpool,
            tc.tile_pool(name="psA", bufs=2, space="PSUM") as psA,
            tc.tile_pool(name="psR", bufs=2, space="PSUM") as psR,
            tc.tile_pool(name="psT", bufs=2, space="PSUM") as psT,
            tc.tile_pool(name="psO", bufs=2, space="PSUM") as psO,
        ):
            gpools = [gpool0, gpool1, gpool2, gpool3]
            nc.gpsimd.load_library(library_config.mlp)

            # one Pool register per distinct num_idxs value (54 regs total)
            _nreg = {}

            def numreg(v):
                if v not in _nreg:
                    r = nc.gpsimd.alloc_register(f"ni_{v}")
                    nc.gpsimd.reg_add(r, 0, v)
                    _nreg[v] = r
                return _nreg[v]

            iota_b = cpool.tile([128, 128], BF16, tag="iota_b")
            nc.sync.dma_start(out=iota_b[:], in_=iota_d[:, :])
            ident = cpool.tile([128, 128], F32, tag="ident")
            nc.sync.dma_start(out=ident[:], in_=ident_d[:, :])
            wt_sb = cpool.tile([128, 2 * OUT_F], F32, tag="wt")
            nc.sync.dma_start(out=wt_sb[:], in_=wt_d[:, :])
            srcl_sb = cpool.tile([128, T_total], BF16, tag="srcl")
            nc.sync.dma_start(out=srcl_sb[:], in_=srcl_d[:, :])
            ones_sb = cpool.tile([128, 1], BF16, tag="ones_sb")
            nc.sync.dma_start(out=ones_sb[:], in_=ones_d[:, :])

            for blocks, per_r in sched:
                # group tile span
                t_g0 = per_r[0][0][1] if per_r[0] else None
                ents_all = [e for ents in per_r for e in ents]
                if not ents_all:
                    continue
                t_g0 = min(e[1] for e in ents_all)
                t_g1 = max(e[1] + e[2] for e in ents_all)
                Tg = t_g1 - t_g0

                lsre = lpool.tile([128, Tg * D_LABEL], F32, tag="lsre")
                nc.sync.dma_start(
                    out=lsre[:],
                    in_=lsre_d[:, t_g0 * D_LABEL : t_g1 * D_LABEL],
                )
                ldste = lpool.tile([128, Tg * D_LABEL], F32, tag="ldste")
                nc.sync.dma_start(
                    out=ldste[:],
                    in_=ldste_d[:, t_g0 * D_LABEL : t_g1 * D_LABEL],
                )
                idx_g = idxpool.tile([128, Tg * 8], I16, tag="idx_g")
                nc.sync.dma_start(
                    out=idx_g[:], in_=idx_d[:, t_g0 * 8 : t_g1 * 8]
                )

                gath = [None] * NRANGE
                expv = [None] * NRANGE
                tr0 = [0] * NRANGE
                for r in range(NRANGE):
                    ents = per_r[r]
                    if not ents:
                        continue
                    T_gr = sum(e[2] for e in ents)
                    tr0[r] = ents[0][1]
                    g = gpools[r].tile([128, T_gr * ROW], BF16, tag=f"gath{r}")
                    gath[r] = (g, T_gr)
                    rbase = r * RNG
                    rend = min(N, (r + 1) * RNG)
                    c0 = (tr0[r] - t_g0) * 8
                    nc.gpsimd.dma_gather(
                        g[:].rearrange("p (t e) -> p t e", e=ROW),
                        tab_d[rbase:rend, :],
                        idx_g[:, c0 : c0 + T_gr * 8],
                        T_gr * BLK,
                        numreg(T_gr * BLK),
                        ROW,
                        # >64 descs per engine overflow the single-packet
                        # coalescing limit and wedge the device
                        single_packet=False,
                        # two queues double the in-flight descriptor rings
                        queue_num=r % 2,
                    )

                # batched attention logits per range (src/dst label edge arrays)
                for r in range(NRANGE):
                    if gath[r] is None:
                        continue
                    g, T_gr = gath[r]
                    loff = (tr0[r] - t_g0) * D_LABEL
                    prod = ppool.tile([128, T_gr * D_LABEL], F32, tag="prod")
                    nc.vector.tensor_tensor(
                        out=prod[:],
                        in0=ldste[:, loff : loff + T_gr * D_LABEL],
                        in1=lsre[:, loff : loff + T_gr * D_LABEL],
                        op=mybir.AluOpType.mult,
                    )
                    dots = dpool.tile([128, T_gr], F32, tag="dots")
                    nc.vector.tensor_reduce(
                        out=dots[:],
                        in_=prod[:].rearrange("p (t k) -> p t k", k=D_LABEL),
                        axis=mybir.AxisListType.X,
                        op=mybir.AluOpType.add,
                    )
                    sc = dpool.tile([128, T_gr], F32, tag="sc")
                    nc.vector.tensor_scalar_mul(sc[:], dots[:], ALPHA)
                    lr = dpool.tile([128, T_gr], F32, tag="lr")
                    nc.vector.tensor_tensor(
                        out=lr[:],
                        in0=sc[:],
                        in1=dots[:],
                        op=mybir.AluOpType.max,
                    )
                    ev = dpool.tile([128, T_gr], BF16, tag="expv")
                    nc.scalar.activation(
                        ev[:], lr[:], mybir.ActivationFunctionType.Exp
                    )
                    expv[r] = ev

                # batched selector build per range: S[p, t*128+q] =
                # (q == srcl[p,t]) * expv[p,t], all bf16
                sall = [None] * NRANGE
                for r in range(NRANGE):
                    if gath[r] is None:
                        continue
                    _, T_gr = gath[r]
                    S_all = spool.tile([128, T_gr * 128], BF16, tag="S_all")
                    iota_bc = (
                        iota_b[:]
                        .rearrange("p (o q) -> p o q", o=1)
                        .to_broadcast([128, T_gr, 128])
                    )
                    srcl_bc = (
                        srcl_sb[:, tr0[r] : tr0[r] + T_gr]
                        .rearrange("p (t o) -> p t o", o=1)
                        .to_broadcast([128, T_gr, 128])
                    )
                    nc.vector.tensor_tensor(
                        out=S_all[:],
                        in0=srcl_bc,
                        in1=iota_bc,
                        op=mybir.AluOpType.is_equal,
                    )
                    ev_bc = (
                        expv[r][:]
                        .rearrange("p (t o) -> p t o", o=1)
                        .to_broadcast([128, T_gr, 128])
                    )
                    nc.vector.tensor_tensor(
                        out=S_all[:],
                        in0=ev_bc,
                        in1=S_all[:].rearrange("p (t q) -> p t q", q=128),
                        op=mybir.AluOpType.mult,
                    )
                    sall[r] = S_all

                # selector matmuls, accumulated per block
                agg = {}
                rs_ps = {}
                first = {b: True for b in blocks}
                last_tile = {}
                for r in range(NRANGE):
                    for b, tb0, nt in per_r[r]:
                        last_tile[b] = (r, tb0 + nt - 1)
                for r in range(NRANGE):
                    if gath[r] is None:
                        continue
                    g, T_gr = gath[r]
                    for b, tb0, nt in per_r[r]:
                        if first.get(b, True):
                            # full-bank tiles: matmul start=True clears the
                            # whole PSUM bank, so concurrent accumulation
                            # groups need exclusive banks
                            agg[b] = psA.tile(
                                [128, 512], F32, tag="agg", name=f"agg{b}"
                            )
                            rs_ps[b] = psR.tile(
                                [128, 512], F32, tag="rs", name=f"rs{b}"
                            )
                        for j in range(nt):
                            t = tb0 + j
                            jj = t - tr0[r]
                            st = first.get(b, True)
                            sp = last_tile[b] == (r, t)
                            nc.tensor.matmul(
                                out=agg[b][:, :IN_F],
                                lhsT=sall[r][:, jj * 128 : (jj + 1) * 128],
                                rhs=g[:, jj * ROW : jj * ROW + IN_F],
                                start=st,
                                stop=sp,
                            )
                            nc.tensor.matmul(
                                out=rs_ps[b][:, 0:1],
                                lhsT=sall[r][:, jj * 128 : (jj + 1) * 128],
                                rhs=ones_sb[:],
                                start=st,
                                stop=sp,
                            )
                            first[b] = False

                # per-block epilogue: normalize + project
                for b in blocks:
                    if b not in agg:
                        continue
                    rows = min(BLK, SHARD - b * BLK)
                    rsm = smpool.tile([128, 1], F32, tag="rsm")
                    nc.vector.tensor_scalar_max(
                        rsm[:], rs_ps[b][:, 0:1], EPS
                    )
                    rcp = smpool.tile([128, 1], F32, tag="rcp")
                    nc.vector.reciprocal(rcp[:], rsm[:])
                    scaled = postpool.tile([128, IN_F], F32, tag="scaled")
                    nc.scalar.activation(
                        scaled[:],
                        agg[b][:, :IN_F],
                        mybir.ActivationFunctionType.Copy,
                        scale=rcp[:, 0:1],
                    )
                    outp = psO.tile(
                        [128, 512], F32, tag="outp", padded_shape=None
                    )
                    for c in range(2):
                        tp = psT.tile([128, 512], F32, tag="tp")
                        nc.tensor.transpose(
                            out=tp[:, 0:128],
                            in_=scaled[:, c * 128 : (c + 1) * 128],
                            identity=ident[:],
                        )
                        sT = postpool.tile([128, 128], F32, tag="sT")
                        nc.scalar.activation(
                            sT[:], tp[:, 0:128], mybir.ActivationFunctionType.Copy
                        )
                        nc.tensor.matmul(
                            out=outp[:, :OUT_F],
                            lhsT=sT[:],
                            rhs=wt_sb[:, c * OUT_F : (c + 1) * OUT_F],
                            start=(c == 0),
                            stop=(c == 1),
                        )
                    osb = postpool.tile([128, OUT_F], F32, tag="osb")
                    nc.scalar.activation(
                        osb[:], outp[:, :OUT_F], mybir.ActivationFunctionType.Copy
                    )
                    nc.sync.dma_start(
                        out=out_d[b * BLK : b * BLK + rows, :], in_=osb[:rows, :]
                    )

    lower_extended_insts(nc)
    return nc


_CACHE = {}


def kernel(h, label, W, adj_indices):
    h = np.asarray(h, dtype=np.float32)
    label = np.asarray(label, dtype=np.float32)
    W = np.asarray(W, dtype=np.float32)
    adj_indices = np.asarray(adj_indices)

    in_maps, sched, T_total, sched_key = _host_prep(h, label, W, adj_indices)

    if sched_key not in _CACHE:
        _CACHE[sched_key] = _build_kernel(sched, T_total)
    nc = _CACHE[sched_key]

    res = run_bass_kernel_spmd(nc, in_maps, core_ids=list(range(NCORES)))
    out = np.concatenate([r["out"] for r in res.results], axis=0)
    return out.astype(np.float32)


# revision 31
# speedup vs baseline: 1.5330x; 1.0074x over previous
"""GAT-style message passing kernel for Trainium2 (8 NeuronCores, SPMD).

h_prime[i] = (sum_j exp(lrelu(<label_i,label_j>)) * h[j]) / rowsum_i @ W

Algebraic transform: W commutes with the segment-sum, so raw h[dst] rows are
aggregated per src node and W applied once per 128-row block.  Per 128-edge
tile a selector matmul (S[p,q] = (q==srcl_p) * exp_p) computes the weighted
sum; a ones column appended to the gathered rows yields the softmax row-sums
in the same matmul.

v2 data path: edge-endpoint rows are fetched with dma_gather (one GPSIMD
instruction per (4-block group, dst-range) instead of one indirect DMA per
tile), from a packed bf16 table row [label(f32-bitcast) | h(bf16) | 1 | pad]
of 768 bytes.  dst ranges of 32768 rows keep gather indices within int16.
Edge src labels are streamed as a host-prepared per-slot array via plain
HWDGE DMA.  Attention logits are computed with DVE ops batched over a whole
gather's tiles.

Sharding: nodes (src) split 12500/core; each core gets all edges whose src it
owns.  One NEFF runs SPMD on all 8 cores; the slot schedule is the per-
(block,range) max across cores so the program is identical.
"""

import sys

sys.path.insert(0, "/opt/trn_rl_repo")

import json

import numpy as np
import ml_dtypes

import concourse.bass as bass
import concourse.mybir as mybir
from concourse import library_config
from concourse.bass_utils import run_bass_kernel_spmd
from concourse.library_overlay import lower_extended_insts

from concourse.tile import TileContext


def _legalize_waits(bir: bytes) -> bytes:
    """This toolchain's codegen allows one sync-wait per instruction; move
    extras onto injected wait-only EventSemaphore ops in the same queue."""
    d = json.loads(bir)
    n = 0
    for fn in d["functions"]:
        for blk in fn["blocks"]:
            out = []
            for inst in blk["instructions"]:
                si = inst.get("sync_info")
                ow = (si or {}).get("on_wait") or []
                if len(ow) > 1:
                    for w in ow[:-1]:
                        n += 1
                        out.append(
                            {
                                "debug": inst.get("debug", 0),
                                "engine": inst.get("engine"),
                                "ins": [],
                                "name": f"waitfix_{n}_{inst['name']}",
                                "opcode": "EventSemaphore",
                                "outs": [],
                                "sync_info": {"on_update": [], "on_wait": [w]},
                            }
                        )
                    si["on_wait"] = [ow[-1]]
                out.append(inst)
            blk["instructions"] = out
    return json.dumps(d).encode()


_orig_to_json_bytes = bass.Bass.to_json_bytes


def _patched_to_json_bytes(self):
    return _legalize_waits(_orig_to_json_bytes(self))


bass.Bass.to_json_bytes = _patched_to_json_bytes

N = 100000
E = 1600000
IN_F = 256
D_LABEL = 32
OUT_F = 256
ALPHA = 0.2
EPS = 1e-9
NCORES = 8
SHARD = N // NCORES          # 12500
BLK = 128
NBLK = (SHARD + BLK - 1) // BLK   # 98
RNG = 32768                  # dst range size (int16 gather indices)
NRANGE = (N + RNG - 1) // RNG     # 4
GRP = 4                      # blocks per gather group
NGRP = (NBLK + GRP - 1) // GRP    # 25
ROW = 256                    # bf16 gather row: h only (512 B, dma_gather-aligned)

F32 = mybir.dt.float32
BF16 = mybir.dt.bfloat16
I16 = mybir.dt.int16


def _host_prep(h, label, W, adj_indices):
    src = np.asarray(adj_indices[0], dtype=np.int64)
    dst = np.asarray(adj_indices[1], dtype=np.int64)

    # bf16 gather table: h rows only (512 B each)
    tab = np.ascontiguousarray(h.astype(ml_dtypes.bfloat16))

    # sort edges once by (core, block, range, dst)
    core = src // SHARD
    blk = (src % SHARD) // BLK
    rng_id = dst // RNG
    key = ((core * NBLK + blk) * NRANGE + rng_id) * np.int64(N) + dst
    order = np.argsort(key, kind="stable")
    s_s, d_s = src[order], dst[order]
    core_s, blk_s, rng_s = core[order], blk[order], rng_id[order]

    # shared schedule: tiles per (block, range) = max over cores
    counts = np.zeros((NCORES, NBLK, NRANGE), dtype=np.int64)
    np.add.at(counts, (core_s, blk_s, rng_s), 1)
    tiles = ((counts + BLK - 1) // BLK).max(axis=0)  # [NBLK, NRANGE]

    # global tile order: (group, range, block in group, tile)
    # tile_start[b, r] = global tile index of (b, r)'s first tile
    tile_start = np.zeros((NBLK, NRANGE), dtype=np.int64)
    t = 0
    sched = []  # per group: (blocks, [(r, [(b, t0, ntile)...])...])
    for g in range(NGRP):
        b0, b1 = g * GRP, min((g + 1) * GRP, NBLK)
        per_r = []
        for r in range(NRANGE):
            ents = []
            for b in range(b0, b1):
                nt = int(tiles[b, r])
                if nt == 0:
                    continue
                tile_start[b, r] = t
                ents.append((b, t, nt))
                t += nt
            per_r.append(ents)
        sched.append((list(range(b0, b1)), per_r))
    T_total = t

    # per-core slot arrays
    C_total = T_total * BLK // 16
    Wt = np.ascontiguousarray(
        np.concatenate([W[:128, :], W[128:, :]], axis=1), dtype=np.float32
    )
    iota_b = np.tile(
        np.arange(128, dtype=np.float32).astype(ml_dtypes.bfloat16), (128, 1)
    )
    ident = np.eye(128, dtype=np.float32)

    # per (core, block, range) run boundaries in the sorted edge list
    run_key = (core_s * NBLK + blk_s) * NRANGE + rng_s
    bounds = np.searchsorted(run_key, np.arange(NCORES * NBLK * NRANGE + 1))

    in_maps = []
    for m in range(NCORES):
        idx16 = np.zeros((128, C_total), dtype=np.int16)
        srcl = np.full((128, T_total), 300.0, dtype=np.float32)
        lsre = np.zeros((128, T_total * D_LABEL), dtype=np.float32)
        ldste = np.zeros((128, T_total * D_LABEL), dtype=np.float32)

        # slot index for every real edge of this core
        for b in range(NBLK):
            for r in range(NRANGE):
                k = (m * NBLK + b) * NRANGE + r
                e0, e1 = bounds[k], bounds[k + 1]
                if e0 == e1:
                    continue
                n_e = e1 - e0
                s0 = tile_start[b, r] * BLK
                sl = s0 + np.arange(n_e)
                dst_loc = (d_s[e0:e1] - r * RNG).astype(np.int16)
                idx16[sl % 16, sl // 16] = dst_loc
                srcl[sl % BLK, sl // BLK] = (s_s[e0:e1] % SHARD - b * BLK).astype(
                    np.float32
                )
                p = sl % BLK
                c0 = (sl // BLK) * D_LABEL
                flat_cols = c0[:, None] + np.arange(D_LABEL)[None, :]
                lsre[p[:, None], flat_cols] = label[s_s[e0:e1]]
                ldste[p[:, None], flat_cols] = label[d_s[e0:e1]]
        # replicas for the rx/tx Q7 cores of queues 0 and 1
        idx16[16:32, :] = idx16[:16, :]
        idx16[32:48, :] = idx16[:16, :]
        idx16[48:64, :] = idx16[:16, :]

        in_maps.append(
            {
                "tab": tab,
                "idx": idx16,
                "srcl": srcl.astype(ml_dtypes.bfloat16),
                "lsre": lsre,
                "ldste": ldste,
                "wt": Wt,
                "iotab": iota_b,
                "identf": ident,
                "onesb": np.ones((128, 1), dtype=ml_dtypes.bfloat16),
            }
        )

    sched_key = tiles.tobytes()
    return in_maps, sched, T_total, sched_key


def _build_kernel(sched, T_total):
    nc = bass.Bass(num_swdge_queues=2)

    C_total = T_total * BLK // 16
    tab_d = nc.dram_tensor("tab", [N, ROW], BF16, kind="ExternalInput")
    idx_d = nc.dram_tensor("idx", [128, C_total], I16, kind="ExternalInput")
    srcl_d = nc.dram_tensor("srcl", [128, T_total], BF16, kind="ExternalInput")
    lsre_d = nc.dram_tensor(
        "lsre", [128, T_total * D_LABEL], F32, kind="ExternalInput"
    )
    ldste_d = nc.dram_tensor(
        "ldste", [128, T_total * D_LABEL], F32, kind="ExternalInput"
    )
    wt_d = nc.dram_tensor("wt", [128, 2 * OUT_F], F32, kind="ExternalInput")
    iota_d = nc.dram_tensor("iotab", [128, 128], BF16, kind="ExternalInput")
    ident_d = nc.dram_tensor("identf", [128, 128], F32, kind="ExternalInput")
    ones_d = nc.dram_tensor("onesb", [128, 1], BF16, kind="ExternalInput")
    out_d = nc.dram_tensor("out", [SHARD, OUT_F], F32, kind="ExternalOutput")

    with TileContext(nc) as tc:
        with (
            tc.tile_pool(name="const", bufs=1) as cpool,
            tc.tile_pool(name="g0", bufs=2) as gpool0,
            tc.tile_pool(name="g1", bufs=2) as gpool1,
            tc.tile_pool(name="g2", bufs=2) as gpool2,
            tc.tile_pool(name="g3", bufs=2) as gpool3,
            tc.tile_pool(name="lsr", bufs=2) as lpool,
            tc.tile_pool(name="idxp", bufs=2) as idxpool,
            tc.tile_pool(name="prod", bufs=3) as ppool,
            tc.tile_pool(name="dots", bufs=10) as dpool,
            tc.tile_pool(name="sel", bufs=6) as spool,
            tc.tile_pool(name="small", bufs=6) as smpool,
            tc.tile_pool(name="post", bufs=3) as postpool,
            tc.tile_pool(name="psA", bufs=4, space="PSUM") as psA,
            tc.tile_pool(name="psT", bufs=2, space="PSUM") as psT,
            tc.tile_pool(name="psO", bufs=2, space="PSUM") as psO,
        ):
            gpools = [gpool0, gpool1, gpool2, gpool3]
            nc.gpsimd.load_library(library_config.mlp)

            # one Pool register per distinct num_idxs value (54 regs total)
            _nreg = {}

            def numreg(v):
                if v not in _nreg:
                    r = nc.gpsimd.alloc_register(f"ni_{v}")
                    nc.gpsimd.reg_add(r, 0, v)
                    _nreg[v] = r
                return _nreg[v]

            iota_b = cpool.tile([128, 128], BF16, tag="iota_b")
            nc.sync.dma_start(out=iota_b[:], in_=iota_d[:, :])
            ident = cpool.tile([128, 128], F32, tag="ident")
            nc.sync.dma_start(out=ident[:], in_=ident_d[:, :])
            wt_sb = cpool.tile([128, 2 * OUT_F], F32, tag="wt")
            nc.sync.dma_start(out=wt_sb[:], in_=wt_d[:, :])
            srcl_sb = cpool.tile([128, T_total], BF16, tag="srcl")
            nc.sync.dma_start(out=srcl_sb[:], in_=srcl_d[:, :])
            ones_sb = cpool.tile([128, 1], BF16, tag="ones_sb")
            nc.sync.dma_start(out=ones_sb[:], in_=ones_d[:, :])

            for blocks, per_r in sched:
                # group tile span
                t_g0 = per_r[0][0][1] if per_r[0] else None
                ents_all = [e for ents in per_r for e in ents]
                if not ents_all:
                    continue
                t_g0 = min(e[1] for e in ents_all)
                t_g1 = max(e[1] + e[2] for e in ents_all)
                Tg = t_g1 - t_g0

                lsre = lpool.tile([128, Tg * D_LABEL], F32, tag="lsre")
                nc.sync.dma_start(
                    out=lsre[:],
                    in_=lsre_d[:, t_g0 * D_LABEL : t_g1 * D_LABEL],
                )
                ldste = lpool.tile([128, Tg * D_LABEL], F32, tag="ldste")
                nc.sync.dma_start(
                    out=ldste[:],
                    in_=ldste_d[:, t_g0 * D_LABEL : t_g1 * D_LABEL],
                )
                idx_g = idxpool.tile([128, Tg * 8], I16, tag="idx_g")
                nc.sync.dma_start(
                    out=idx_g[:], in_=idx_d[:, t_g0 * 8 : t_g1 * 8]
                )

                gath = [None] * NRANGE
                expv = [None] * NRANGE
                tr0 = [0] * NRANGE
                for r in range(NRANGE):
                    ents = per_r[r]
                    if not ents:
                        continue
                    T_gr = sum(e[2] for e in ents)
                    tr0[r] = ents[0][1]
                    g = gpools[r].tile([128, T_gr * ROW], BF16, tag=f"gath{r}")
                    gath[r] = (g, T_gr)
                    rbase = r * RNG
                    rend = min(N, (r + 1) * RNG)
                    c0 = (tr0[r] - t_g0) * 8
                    nc.gpsimd.dma_gather(
                        g[:].rearrange("p (t e) -> p t e", e=ROW),
                        tab_d[rbase:rend, :],
                        idx_g[:, c0 : c0 + T_gr * 8],
                        T_gr * BLK,
                        numreg(T_gr * BLK),
                        ROW,
                        # >64 descs per engine overflow the single-packet
                        # coalescing limit and wedge the device
                        single_packet=False,
                        # two queues double the in-flight descriptor rings
                        queue_num=r % 2,
                    )

                # batched attention logits per range (src/dst label edge arrays)
                for r in range(NRANGE):
                    if gath[r] is None:
                        continue
                    g, T_gr = gath[r]
                    loff = (tr0[r] - t_g0) * D_LABEL
                    prod = ppool.tile([128, T_gr * D_LABEL], F32, tag="prod")
                    nc.vector.tensor_tensor(
                        out=prod[:],
                        in0=ldste[:, loff : loff + T_gr * D_LABEL],
                        in1=lsre[:, loff : loff + T_gr * D_LABEL],
                        op=mybir.AluOpType.mult,
                    )
                    dots = dpool.tile([128, T_gr], F32, tag="dots")
                    nc.vector.tensor_reduce(
                        out=dots[:],
                        in_=prod[:].rearrange("p (t k) -> p t k", k=D_LABEL),
                        axis=mybir.AxisListType.X,
                        op=mybir.AluOpType.add,
                    )
                    sc = dpool.tile([128, T_gr], F32, tag="sc")
                    nc.vector.tensor_scalar_mul(sc[:], dots[:], ALPHA)
                    lr = dpool.tile([128, T_gr], F32, tag="lr")
                    nc.vector.tensor_tensor(
                        out=lr[:],
                        in0=sc[:],
                        in1=dots[:],
                        op=mybir.AluOpType.max,
                    )
                    ev = dpool.tile([128, T_gr], BF16, tag="expv")
                    nc.scalar.activation(
                        ev[:], lr[:], mybir.ActivationFunctionType.Exp
                    )
                    expv[r] = ev

                # batched selector build per range: S[p, t*128+q] =
                # (q == srcl[p,t]) * expv[p,t], all bf16
                sall = [None] * NRANGE
                for r in range(NRANGE):
                    if gath[r] is None:
                        continue
                    _, T_gr = gath[r]
                    S_all = spool.tile([128, T_gr * 128], BF16, tag="S_all")
                    iota_bc = (
                        iota_b[:]
                        .rearrange("p (o q) -> p o q", o=1)
                        .to_broadcast([128, T_gr, 128])
                    )
                    srcl_bc = (
                        srcl_sb[:, tr0[r] : tr0[r] + T_gr]
                        .rearrange("p (t o) -> p t o", o=1)
                        .to_broadcast([128, T_gr, 128])
                    )
                    nc.vector.tensor_tensor(
                        out=S_all[:],
                        in0=srcl_bc,
                        in1=iota_bc,
                        op=mybir.AluOpType.is_equal,
                    )
                    ev_bc = (
                        expv[r][:]
                        .rearrange("p (t o) -> p t o", o=1)
                        .to_broadcast([128, T_gr, 128])
                    )
                    nc.vector.tensor_tensor(
                        out=S_all[:],
                        in0=ev_bc,
                        in1=S_all[:].rearrange("p (t q) -> p t q", q=128),
                        op=mybir.AluOpType.mult,
                    )
                    sall[r] = S_all

                # selector matmuls, accumulated per block
                agg = {}
                first = {b: True for b in blocks}
                last_tile = {}
                for r in range(NRANGE):
                    for b, tb0, nt in per_r[r]:
                        last_tile[b] = (r, tb0 + nt - 1)
                for r in range(NRANGE):
                    if gath[r] is None:
                        continue
                    g, T_gr = gath[r]
                    for b, tb0, nt in per_r[r]:
                        if first.get(b, True):
                            # full-bank tile: matmul start=True clears the
                            # whole PSUM bank, so concurrent accumulation
                            # groups need exclusive banks
                            agg[b] = psA.tile(
                                [128, 512], F32, tag="agg", name=f"agg{b}"
                            )
                        for j in range(nt):
                            t = tb0 + j
                            jj = t - tr0[r]
                            st = first.get(b, True)
                            sp = last_tile[b] == (r, t)
                            nc.tensor.matmul(
                                out=agg[b][:, :IN_F],
                                lhsT=sall[r][:, jj * 128 : (jj + 1) * 128],
                                rhs=g[:, jj * ROW : jj * ROW + IN_F],
                                start=st,
                                stop=sp,
                            )
                            first[b] = False

                # per-block epilogue: drain agg, rowsum into the freed bank,
                # project, normalize at the final copy
                for b in blocks:
                    if b not in agg:
                        continue
                    rows = min(BLK, SHARD - b * BLK)
                    scaled = postpool.tile([128, IN_F], F32, tag="scaled")
                    nc.scalar.activation(
                        scaled[:],
                        agg[b][:, :IN_F],
                        mybir.ActivationFunctionType.Copy,
                    )
                    # rowsum accumulation reuses the agg bank; its start=True
                    # clears the bank, but the copy above already drained it
                    tiles_b = [
                        (r, t)
                        for r in range(NRANGE)
                        for (bb, tb0, nt) in per_r[r]
                        if bb == b
                        for t in range(tb0, tb0 + nt)
                    ]
                    for k, (r, t) in enumerate(tiles_b):
                        jj = t - tr0[r]
                        nc.tensor.matmul(
                            out=agg[b][:, 0:1],
                            lhsT=sall[r][:, jj * 128 : (jj + 1) * 128],
                            rhs=ones_sb[:],
                            start=(k == 0),
                            stop=(k == len(tiles_b) - 1),
                        )
                    rsm = smpool.tile([128, 1], F32, tag="rsm")
                    nc.vector.tensor_scalar_max(
                        rsm[:], agg[b][:, 0:1], EPS
                    )
                    rcp = smpool.tile([128, 1], F32, tag="rcp")
                    nc.vector.reciprocal(rcp[:], rsm[:])
                    outp = psO.tile(
                        [128, 512], F32, tag="outp", padded_shape=None
                    )
                    for c in range(2):
                        tp = psT.tile([128, 512], F32, tag="tp")
                        nc.tensor.transpose(
                            out=tp[:, 0:128],
                            in_=scaled[:, c * 128 : (c + 1) * 128],
                            identity=ident[:],
                        )
                        sT = postpool.tile([128, 128], F32, tag="sT")
                        nc.scalar.activation(
                            sT[:], tp[:, 0:128], mybir.ActivationFunctionType.Copy
                        )
                        nc.tensor.matmul(
                            out=outp[:, :OUT_F],
                            lhsT=sT[:],
                            rhs=wt_sb[:, c * OUT_F : (c + 1) * OUT_F],
                            start=(c == 0),
                            stop=(c == 1),
                        )
                    osb = postpool.tile([128, OUT_F], F32, tag="osb")
                    nc.scalar.activation(
                        osb[:],
                        outp[:, :OUT_F],
                        mybir.ActivationFunctionType.Copy,
                        scale=rcp[:, 0:1],
                    )
                    nc.sync.dma_start(
                        out=out_d[b * BLK : b * BLK + rows, :], in_=osb[:rows, :]
                    )

    lower_extended_insts(nc)
    return nc


_CACHE = {}


def kernel(h, label, W, adj_indices):
    h = np.asarray(h, dtype=np.float32)
    label = np.asarray(label, dtype=np.float32)
    W = np.asarray(W, dtype=np.float32)
    adj_indices = np.asarray(adj_indices)

    in_maps, sched, T_total, sched_key = _host_prep(h, label, W, adj_indices)

    if sched_key not in _CACHE:
        _CACHE[sched_key] = _build_kernel(sched, T_total)
    nc = _CACHE[sched_key]

    res = run_bass_kernel_spmd(nc, in_maps, core_ids=list(range(NCORES)))
    out = np.concatenate([r["out"] for r in res.results], axis=0)
    return out.astype(np.float32)


# revision 32
# speedup vs baseline: 1.8068x; 1.1786x over previous
"""GAT-style message passing kernel for Trainium2 (8 NeuronCores, SPMD).

h_prime[i] = (sum_j exp(lrelu(<label_i,label_j>)) * h[j]) / rowsum_i @ W

Algebraic transform: W commutes with the segment-sum, so raw h[dst] rows are
aggregated per src node and W applied once per 128-row block.  Per 128-edge
tile a selector matmul (S[p,q] = (q==srcl_p) * exp_p) computes the weighted
sum; a ones column appended to the gathered rows yields the softmax row-sums
in the same matmul.

v2 data path: edge-endpoint rows are fetched with dma_gather (one GPSIMD
instruction per (4-block group, dst-range) instead of one indirect DMA per
tile), from a packed bf16 table row [label(f32-bitcast) | h(bf16) | 1 | pad]
of 768 bytes.  dst ranges of 32768 rows keep gather indices within int16.
Edge src labels are streamed as a host-prepared per-slot array via plain
HWDGE DMA.  Attention logits are computed with DVE ops batched over a whole
gather's tiles.

Sharding: nodes (src) split 12500/core; each core gets all edges whose src it
owns.  One NEFF runs SPMD on all 8 cores; the slot schedule is the per-
(block,range) max across cores so the program is identical.
"""

import sys

sys.path.insert(0, "/opt/trn_rl_repo")

import json

import numpy as np
import ml_dtypes

import concourse.bass as bass
import concourse.mybir as mybir
from concourse import library_config
from concourse.bass_utils import run_bass_kernel_spmd
from concourse.library_overlay import lower_extended_insts

from concourse.tile import TileContext


def _legalize_waits(bir: bytes) -> bytes:
    """This toolchain's codegen allows one sync-wait per instruction; move
    extras onto injected wait-only EventSemaphore ops in the same queue."""
    d = json.loads(bir)
    n = 0
    for fn in d["functions"]:
        for blk in fn["blocks"]:
            out = []
            for inst in blk["instructions"]:
                si = inst.get("sync_info")
                ow = (si or {}).get("on_wait") or []
                if len(ow) > 1:
                    for w in ow[:-1]:
                        n += 1
                        out.append(
                            {
                                "debug": inst.get("debug", 0),
                                "engine": inst.get("engine"),
                                "ins": [],
                                "name": f"waitfix_{n}_{inst['name']}",
                                "opcode": "EventSemaphore",
                                "outs": [],
                                "sync_info": {"on_update": [], "on_wait": [w]},
                            }
                        )
                    si["on_wait"] = [ow[-1]]
                out.append(inst)
            blk["instructions"] = out
    return json.dumps(d).encode()


_orig_to_json_bytes = bass.Bass.to_json_bytes


def _patched_to_json_bytes(self):
    return _legalize_waits(_orig_to_json_bytes(self))


bass.Bass.to_json_bytes = _patched_to_json_bytes

N = 100000
E = 1600000
IN_F = 256
D_LABEL = 32
OUT_F = 256
ALPHA = 0.2
EPS = 1e-9
NCORES = 8
SHARD = N // NCORES          # 12500
BLK = 128
NBLK = (SHARD + BLK - 1) // BLK   # 98
RNG = 32768                  # dst range size (int16 gather indices)
NRANGE = (N + RNG - 1) // RNG     # 4
GRP = 4                      # blocks per gather group
NGRP = (NBLK + GRP - 1) // GRP    # 25
ROW = 256                    # bf16 gather row: h only (512 B, dma_gather-aligned)

F32 = mybir.dt.float32
BF16 = mybir.dt.bfloat16
I16 = mybir.dt.int16


def _host_prep(h, label, W, adj_indices):
    src = np.asarray(adj_indices[0], dtype=np.int64)
    dst = np.asarray(adj_indices[1], dtype=np.int64)

    # bf16 gather table: h rows only (512 B each)
    tab = np.ascontiguousarray(h.astype(ml_dtypes.bfloat16))

    # sort edges once by (core, block, range, dst)
    core = src // SHARD
    blk = (src % SHARD) // BLK
    rng_id = dst // RNG
    key = ((core * NBLK + blk) * NRANGE + rng_id) * np.int64(N) + dst
    order = np.argsort(key, kind="stable")
    s_s, d_s = src[order], dst[order]
    core_s, blk_s, rng_s = core[order], blk[order], rng_id[order]

    # shared schedule: tiles per (block, range) = max over cores
    counts = np.zeros((NCORES, NBLK, NRANGE), dtype=np.int64)
    np.add.at(counts, (core_s, blk_s, rng_s), 1)
    tiles = ((counts + BLK - 1) // BLK).max(axis=0)  # [NBLK, NRANGE]

    # global tile order: (group, range, block in group, tile)
    # tile_start[b, r] = global tile index of (b, r)'s first tile
    tile_start = np.zeros((NBLK, NRANGE), dtype=np.int64)
    t = 0
    sched = []  # per group: (blocks, [(r, [(b, t0, ntile)...])...])
    for g in range(NGRP):
        b0, b1 = g * GRP, min((g + 1) * GRP, NBLK)
        per_r = []
        for r in range(NRANGE):
            ents = []
            for b in range(b0, b1):
                nt = int(tiles[b, r])
                if nt == 0:
                    continue
                tile_start[b, r] = t
                ents.append((b, t, nt))
                t += nt
            per_r.append(ents)
        sched.append((list(range(b0, b1)), per_r))
    T_total = t

    # per-core slot arrays
    C_total = T_total * BLK // 16
    Wt = np.ascontiguousarray(
        np.concatenate([W[:128, :], W[128:, :]], axis=1), dtype=np.float32
    )
    iota_b = np.tile(
        np.arange(128, dtype=np.float32).astype(ml_dtypes.bfloat16), (128, 1)
    )
    ident = np.eye(128, dtype=np.float32)

    # per (core, block, range) run boundaries in the sorted edge list
    run_key = (core_s * NBLK + blk_s) * NRANGE + rng_s
    bounds = np.searchsorted(run_key, np.arange(NCORES * NBLK * NRANGE + 1))

    in_maps = []
    for m in range(NCORES):
        idx16 = np.zeros((128, C_total), dtype=np.int16)
        srcl = np.full((128, T_total), 300.0, dtype=np.float32)
        lsre = np.zeros((128, T_total * D_LABEL), dtype=np.float32)
        ldste = np.zeros((128, T_total * D_LABEL), dtype=np.float32)

        # slot index for every real edge of this core
        for b in range(NBLK):
            for r in range(NRANGE):
                k = (m * NBLK + b) * NRANGE + r
                e0, e1 = bounds[k], bounds[k + 1]
                if e0 == e1:
                    continue
                n_e = e1 - e0
                s0 = tile_start[b, r] * BLK
                sl = s0 + np.arange(n_e)
                dst_loc = (d_s[e0:e1] - r * RNG).astype(np.int16)
                idx16[sl % 16, sl // 16] = dst_loc
                srcl[sl % BLK, sl // BLK] = (s_s[e0:e1] % SHARD - b * BLK).astype(
                    np.float32
                )
                p = sl % BLK
                c0 = (sl // BLK) * D_LABEL
                flat_cols = c0[:, None] + np.arange(D_LABEL)[None, :]
                lsre[p[:, None], flat_cols] = label[s_s[e0:e1]]
                ldste[p[:, None], flat_cols] = label[d_s[e0:e1]]
        # replicas for the rx/tx Q7 cores of queues 0 and 1
        idx16[16:32, :] = idx16[:16, :]
        idx16[32:48, :] = idx16[:16, :]
        idx16[48:64, :] = idx16[:16, :]

        in_maps.append(
            {
                "tab": tab,
                "idx": idx16,
                "srcl": srcl.astype(ml_dtypes.bfloat16),
                "lsre": lsre,
                "ldste": ldste,
                "wt": Wt,
                "iotab": iota_b,
                "identf": ident,
                "onesb": np.ones((128, 1), dtype=ml_dtypes.bfloat16),
            }
        )

    sched_key = tiles.tobytes()
    return in_maps, sched, T_total, sched_key


def _build_kernel(sched, T_total):
    nc = bass.Bass(num_swdge_queues=2)

    C_total = T_total * BLK // 16
    tab_d = nc.dram_tensor("tab", [N, ROW], BF16, kind="ExternalInput")
    idx_d = nc.dram_tensor("idx", [128, C_total], I16, kind="ExternalInput")
    srcl_d = nc.dram_tensor("srcl", [128, T_total], BF16, kind="ExternalInput")
    lsre_d = nc.dram_tensor(
        "lsre", [128, T_total * D_LABEL], F32, kind="ExternalInput"
    )
    ldste_d = nc.dram_tensor(
        "ldste", [128, T_total * D_LABEL], F32, kind="ExternalInput"
    )
    wt_d = nc.dram_tensor("wt", [128, 2 * OUT_F], F32, kind="ExternalInput")
    iota_d = nc.dram_tensor("iotab", [128, 128], BF16, kind="ExternalInput")
    ident_d = nc.dram_tensor("identf", [128, 128], F32, kind="ExternalInput")
    ones_d = nc.dram_tensor("onesb", [128, 1], BF16, kind="ExternalInput")
    out_d = nc.dram_tensor("out", [SHARD, OUT_F], F32, kind="ExternalOutput")

    with TileContext(nc) as tc:
        with (
            tc.tile_pool(name="const", bufs=1) as cpool,
            tc.tile_pool(name="g0", bufs=2) as gpool0,
            tc.tile_pool(name="g1", bufs=2) as gpool1,
            tc.tile_pool(name="g2", bufs=2) as gpool2,
            tc.tile_pool(name="g3", bufs=2) as gpool3,
            tc.tile_pool(name="lsr", bufs=2) as lpool,
            tc.tile_pool(name="idxp", bufs=2) as idxpool,
            tc.tile_pool(name="prod", bufs=3) as ppool,
            tc.tile_pool(name="dots", bufs=10) as dpool,
            tc.tile_pool(name="sel", bufs=6) as spool,
            tc.tile_pool(name="small", bufs=6) as smpool,
            tc.tile_pool(name="post", bufs=3) as postpool,
            tc.tile_pool(name="psA", bufs=4, space="PSUM") as psA,
            tc.tile_pool(name="psT", bufs=2, space="PSUM") as psT,
            tc.tile_pool(name="psO", bufs=2, space="PSUM") as psO,
        ):
            gpools = [gpool0, gpool1, gpool2, gpool3]
            nc.gpsimd.load_library(library_config.mlp)

            # one Pool register per distinct num_idxs value (54 regs total)
            _nreg = {}

            def numreg(v):
                if v not in _nreg:
                    r = nc.gpsimd.alloc_register(f"ni_{v}")
                    nc.gpsimd.reg_add(r, 0, v)
                    _nreg[v] = r
                return _nreg[v]

            iota_b = cpool.tile([128, 128], BF16, tag="iota_b")
            nc.sync.dma_start(out=iota_b[:], in_=iota_d[:, :])
            ident = cpool.tile([128, 128], F32, tag="ident")
            nc.sync.dma_start(out=ident[:], in_=ident_d[:, :])
            wt_sb = cpool.tile([128, 2 * OUT_F], F32, tag="wt")
            nc.sync.dma_start(out=wt_sb[:], in_=wt_d[:, :])
            srcl_sb = cpool.tile([128, T_total], BF16, tag="srcl")
            nc.sync.dma_start(out=srcl_sb[:], in_=srcl_d[:, :])
            ones_sb = cpool.tile([128, 1], BF16, tag="ones_sb")
            nc.sync.dma_start(out=ones_sb[:], in_=ones_d[:, :])

            for blocks, per_r in sched:
                # group tile span
                t_g0 = per_r[0][0][1] if per_r[0] else None
                ents_all = [e for ents in per_r for e in ents]
                if not ents_all:
                    continue
                t_g0 = min(e[1] for e in ents_all)
                t_g1 = max(e[1] + e[2] for e in ents_all)
                Tg = t_g1 - t_g0

                lsre = lpool.tile([128, Tg * D_LABEL], F32, tag="lsre")
                nc.sync.dma_start(
                    out=lsre[:],
                    in_=lsre_d[:, t_g0 * D_LABEL : t_g1 * D_LABEL],
                )
                ldste = lpool.tile([128, Tg * D_LABEL], F32, tag="ldste")
                nc.sync.dma_start(
                    out=ldste[:],
                    in_=ldste_d[:, t_g0 * D_LABEL : t_g1 * D_LABEL],
                )
                idx_g = idxpool.tile([128, Tg * 8], I16, tag="idx_g")
                nc.sync.dma_start(
                    out=idx_g[:], in_=idx_d[:, t_g0 * 8 : t_g1 * 8]
                )

                gath = [None] * NRANGE
                expv = [None] * NRANGE
                tr0 = [0] * NRANGE
                Tgr_of = [0] * NRANGE
                for r in range(NRANGE):
                    ents = per_r[r]
                    if not ents:
                        continue
                    Tgr_of[r] = sum(e[2] for e in ents)
                    tr0[r] = ents[0][1]

                # selector masks depend only on srcl: build them up front so
                # the DVE has ready work while the gathers are in flight
                sall = [None] * NRANGE
                for r in range(NRANGE):
                    T_gr = Tgr_of[r]
                    if T_gr == 0:
                        continue
                    S_all = spool.tile([128, T_gr * 128], BF16, tag="S_all")
                    iota_bc = (
                        iota_b[:]
                        .rearrange("p (o q) -> p o q", o=1)
                        .to_broadcast([128, T_gr, 128])
                    )
                    srcl_bc = (
                        srcl_sb[:, tr0[r] : tr0[r] + T_gr]
                        .rearrange("p (t o) -> p t o", o=1)
                        .to_broadcast([128, T_gr, 128])
                    )
                    nc.vector.tensor_tensor(
                        out=S_all[:],
                        in0=srcl_bc,
                        in1=iota_bc,
                        op=mybir.AluOpType.is_equal,
                    )
                    sall[r] = S_all

                for r in range(NRANGE):
                    ents = per_r[r]
                    if not ents:
                        continue
                    T_gr = Tgr_of[r]
                    g = gpools[r].tile([128, T_gr * ROW], BF16, tag=f"gath{r}")
                    gath[r] = (g, T_gr)
                    rbase = r * RNG
                    rend = min(N, (r + 1) * RNG)
                    c0 = (tr0[r] - t_g0) * 8
                    nc.gpsimd.dma_gather(
                        g[:].rearrange("p (t e) -> p t e", e=ROW),
                        tab_d[rbase:rend, :],
                        idx_g[:, c0 : c0 + T_gr * 8],
                        T_gr * BLK,
                        numreg(T_gr * BLK),
                        ROW,
                        # >64 descs per engine overflow the single-packet
                        # coalescing limit and wedge the device
                        single_packet=False,
                        # two queues double the in-flight descriptor rings
                        queue_num=r % 2,
                    )

                # batched attention logits per range (src/dst label edge arrays)
                for r in range(NRANGE):
                    if gath[r] is None:
                        continue
                    g, T_gr = gath[r]
                    loff = (tr0[r] - t_g0) * D_LABEL
                    prod = ppool.tile([128, T_gr * D_LABEL], F32, tag="prod")
                    nc.vector.tensor_tensor(
                        out=prod[:],
                        in0=ldste[:, loff : loff + T_gr * D_LABEL],
                        in1=lsre[:, loff : loff + T_gr * D_LABEL],
                        op=mybir.AluOpType.mult,
                    )
                    dots = dpool.tile([128, T_gr], F32, tag="dots")
                    nc.vector.tensor_reduce(
                        out=dots[:],
                        in_=prod[:].rearrange("p (t k) -> p t k", k=D_LABEL),
                        axis=mybir.AxisListType.X,
                        op=mybir.AluOpType.add,
                    )
                    e1 = dpool.tile([128, T_gr], F32, tag="e1")
                    nc.scalar.activation(
                        e1[:], dots[:], mybir.ActivationFunctionType.Exp
                    )
                    e2 = dpool.tile([128, T_gr], F32, tag="e2")
                    nc.scalar.activation(
                        e2[:],
                        dots[:],
                        mybir.ActivationFunctionType.Exp,
                        scale=ALPHA,
                    )
                    # exp is monotone: exp(lrelu(x)) == max(exp(x), exp(a*x))
                    ev = dpool.tile([128, T_gr], BF16, tag="expv")
                    nc.vector.tensor_tensor(
                        out=ev[:],
                        in0=e1[:],
                        in1=e2[:],
                        op=mybir.AluOpType.max,
                    )
                    expv[r] = ev

                # scale the prebuilt masks by expv (bf16)
                for r in range(NRANGE):
                    if gath[r] is None or sall[r] is None:
                        continue
                    _, T_gr = gath[r]
                    ev_bc = (
                        expv[r][:]
                        .rearrange("p (t o) -> p t o", o=1)
                        .to_broadcast([128, T_gr, 128])
                    )
                    nc.vector.tensor_tensor(
                        out=sall[r][:],
                        in0=ev_bc,
                        in1=sall[r][:].rearrange("p (t q) -> p t q", q=128),
                        op=mybir.AluOpType.mult,
                    )

                # selector matmuls, accumulated per block
                agg = {}
                first = {b: True for b in blocks}
                last_tile = {}
                for r in range(NRANGE):
                    for b, tb0, nt in per_r[r]:
                        last_tile[b] = (r, tb0 + nt - 1)
                for r in range(NRANGE):
                    if gath[r] is None:
                        continue
                    g, T_gr = gath[r]
                    for b, tb0, nt in per_r[r]:
                        if first.get(b, True):
                            # full-bank tile: matmul start=True clears the
                            # whole PSUM bank, so concurrent accumulation
                            # groups need exclusive banks
                            agg[b] = psA.tile(
                                [128, 512], F32, tag="agg", name=f"agg{b}"
                            )
                        for j in range(nt):
                            t = tb0 + j
                            jj = t - tr0[r]
                            st = first.get(b, True)
                            sp = last_tile[b] == (r, t)
                            nc.tensor.matmul(
                                out=agg[b][:, :IN_F],
                                lhsT=sall[r][:, jj * 128 : (jj + 1) * 128],
                                rhs=g[:, jj * ROW : jj * ROW + IN_F],
                                start=st,
                                stop=sp,
                            )
                            first[b] = False

                # per-block epilogue: drain agg, rowsum into the freed bank,
                # project, normalize at the final copy
                for b in blocks:
                    if b not in agg:
                        continue
                    rows = min(BLK, SHARD - b * BLK)
                    scaled = postpool.tile([128, IN_F], F32, tag="scaled")
                    nc.scalar.activation(
                        scaled[:],
                        agg[b][:, :IN_F],
                        mybir.ActivationFunctionType.Copy,
                    )
                    # rowsum accumulation reuses the agg bank; its start=True
                    # clears the bank, but the copy above already drained it
                    tiles_b = [
                        (r, t)
                        for r in range(NRANGE)
                        for (bb, tb0, nt) in per_r[r]
                        if bb == b
                        for t in range(tb0, tb0 + nt)
                    ]
                    for k, (r, t) in enumerate(tiles_b):
                        jj = t - tr0[r]
                        nc.tensor.matmul(
                            out=agg[b][:, 0:1],
                            lhsT=sall[r][:, jj * 128 : (jj + 1) * 128],
                            rhs=ones_sb[:],
                            start=(k == 0),
                            stop=(k == len(tiles_b) - 1),
                        )
                    rsm = smpool.tile([128, 1], F32, tag="rsm")
                    nc.vector.tensor_scalar_max(
                        rsm[:], agg[b][:, 0:1], EPS
                    )
                    rcp = smpool.tile([128, 1], F32, tag="rcp")
                    nc.vector.reciprocal(rcp[:], rsm[:])
                    outp = psO.tile(
                        [128, 512], F32, tag="outp", padded_shape=None
                    )
                    for c in range(2):
                        tp = psT.tile([128, 512], F32, tag="tp")
                        nc.tensor.transpose(
                            out=tp[:, 0:128],
                            in_=scaled[:, c * 128 : (c + 1) * 128],
                            identity=ident[:],
                        )
                        sT = postpool.tile([128, 128], F32, tag="sT")
                        nc.scalar.activation(
                            sT[:], tp[:, 0:128], mybir.ActivationFunctionType.Copy
                        )
                        nc.tensor.matmul(
                            out=outp[:, :OUT_F],
                            lhsT=sT[:],
                            rhs=wt_sb[:, c * OUT_F : (c + 1) * OUT_F],
                            start=(c == 0),
                            stop=(c == 1),
                        )
                    osb = postpool.tile([128, OUT_F], F32, tag="osb")
                    nc.scalar.activation(
                        osb[:],
                        outp[:, :OUT_F],
                        mybir.ActivationFunctionType.Copy,
                        scale=rcp[:, 0:1],
                    )
                    nc.sync.dma_start(
                        out=out_d[b * BLK : b * BLK + rows, :], in_=osb[:rows, :]
                    )

    lower_extended_insts(nc)
    return nc


_CACHE = {}


def kernel(h, label, W, adj_indices):
    h = np.asarray(h, dtype=np.float32)
    label = np.asarray(label, dtype=np.float32)
    W = np.asarray(W, dtype=np.float32)
    adj_indices = np.asarray(adj_indices)

    in_maps, sched, T_total, sched_key = _host_prep(h, label, W, adj_indices)

    if sched_key not in _CACHE:
        _CACHE[sched_key] = _build_kernel(sched, T_total)
    nc = _CACHE[sched_key]

    res = run_bass_kernel_spmd(nc, in_maps, core_ids=list(range(NCORES)))
    out = np.concatenate([r["out"] for r in res.results], axis=0)
    return out.astype(np.float32)
